# revision 1
# baseline (speedup 1.0000x reference)
"""Trainium2 Bass kernel for nn_CNLinkPredictor (gnn_message_passing).

Strategy: data-parallel over target edges T (8192) across 8 NeuronCores
(1024 edges/core). Per core, the per-edge CN-token transformer is computed
in tiles of 4 edges = 128 tokens (tokens on SBUF partitions).

Layout/algorithm notes:
 - pf = [xw|xi|xj|xi*xj] @ tok_w.T is split: the xw part is a per-token
   matmul; the (xi,xj,xi*xj) part depends only on the edge -> computed once
   per edge ("EC") and broadcast to the edge's 32 tokens via a rank-4 matmul.
 - LayerNorm affine transforms (gamma/beta) are folded into the following
   matmul weights on the host; device LN computes only (x-mu)*rsqrt(var+eps).
 - qT/kT are produced channel-major with heads padded to 32-partition slots
   so score matmuls can use 32-aligned lhsT partition slices.
 - scores land in PSUM [128 ktok, 8 heads x 128 qtok]; masking (block-diag
   cross-edge + key-padding) is folded into the exp's per-partition bias
   (4 activation calls, one per edge in the tile).
 - softmax denominators ride along as a 17th column of V ("aug-V"); ctx
   matmuls produce [17, 128] blocks (16 ctx channels + denominator row) in
   32-partition head slots; normalization = stream_shuffle + divide.
 - v bias, out_proj bias, ff biases etc. are folded on host where linear.
"""

import sys
import threading

sys.path.insert(0, "/opt/trn_rl_repo")

import numpy as np

import concourse.bass as bass
import concourse.bacc as bacc
import concourse.mybir as mybir
from concourse.tile import TileContext
from concourse.masks import make_identity
from concourse.bass_utils import run_bass_kernel_spmd

F32 = mybir.dt.float32
I32 = mybir.dt.int32
AF = mybir.ActivationFunctionType
ALU = mybir.AluOpType

N, C, H, O = 100000, 128, 256, 1
T, K = 8192, 32
NHEAD, DH, FF = 8, 16, 512
NCORES = 8
TC = T // NCORES          # 1024 edges per core
NT = TC * K // 128        # 256 main tiles (4 edges / 128 tokens each)
NE = TC // 128            # 8 edge tiles (phase A)
NEG = -1e9
EPS_DENOM = 1e-30


def _build_nc(nt=NT, phases="abc", bsteps=99):
    assert nt % 32 == 0
    tcn = 4 * nt           # edges covered by this build
    ne = tcn // 128        # phase-A tiles
    nc = bacc.Bacc("TRN2", target_bir_lowering=False)

    dt = {}

    def din(name, shape, dtype=F32):
        dt[name] = nc.dram_tensor(name, shape, dtype, kind="ExternalInput")
        return dt[name]

    # data
    din("x", [N, C])
    din("idx_cn", [128, nt], I32)
    din("idx_t0", [128, ne], I32)
    din("idx_t1", [128, ne], I32)
    din("valid", [128, nt])
    din("vmbd", [128, 4 * nt])
    din("ebd", [128, 4 * nt])
    din("ind", [1, 4 * nt])
    # weights / constants
    for nm in ["w0xT", "a1", "a2", "a3", "wk_l",
               "woutA", "woutB"] + [f"wqbd_{h}" for h in range(8)]:
        din(nm, [128, 128])
    din("wv_aug", [128, 129])
    for nm in ["wff1_0", "wff1_1", "wff1_2", "wff1_3",
               "wff2_0", "wff2_1", "wff2_2", "wff2_3"]:
        din(nm, [128, 128])
    for nm in ["wx1_0", "wx1_1", "wxj1_0", "wxj1_1"]:
        din(nm, [128, 128])
    for ic in range(2):
        for oc in range(2):
            din(f"wx2_{ic}{oc}", [128, 128])
            din(f"wx3_{ic}{oc}", [128, 128])
            din(f"wxj2_{ic}{oc}", [128, 128])
            din(f"wl1_{ic}{oc}", [128, 128])
    din("wl2_0", [128, 1])
    din("wl2_1", [128, 1])
    din("sel4", [4, 128])
    din("ones1", [1, 128])
    din("tokb_row", [1, 128])
    din("outb_row", [1, 128])
    din("bff2_row", [1, 128])
    for nm in ["beta_col", "eps_col", "epsd_col"]:
        din(nm, [128, 1])
    for nm in ["bff1_0", "bff1_1", "bff1_2", "bff1_3",
               "bx1_0", "bx1_1", "bx2_0", "bx2_1", "bx3_0", "bx3_1",
               "bxj1_0", "bxj1_1", "bxj2_0", "bxj2_1", "bl1_0", "bl1_1"]:
        din(nm, [128, 1])
    din("bl2", [1, 1])

    ec_dram = nc.dram_tensor("ec_dram", [tcn, 128], F32)  # internal scratch
    xcn_dram = nc.dram_tensor("xcn_dram", [tcn, 128], F32)
    out_dram = nc.dram_tensor("out", [1, 4 * nt], F32, kind="ExternalOutput")

    with TileContext(nc) as tc:
        with (
            tc.tile_pool(name="cpool", bufs=1) as cp,
            tc.tile_pool(name="wpool", bufs=3) as wp,
            tc.tile_pool(name="mlppool", bufs=1) as mp,
            tc.tile_pool(name="carry", bufs=18) as cr,
            tc.tile_pool(name="ps", bufs=2, space="PSUM") as ps,
            tc.tile_pool(name="psbig", bufs=4, space="PSUM") as psb,
            tc.tile_pool(name="psctx", bufs=2, space="PSUM") as psc,
        ):
            # ---- load constants to SBUF --------------------------------
            cs = {}
            for nm, t in dt.items():
                if nm == "x":
                    continue
                tile = cp.tile(list(t.shape), t.dtype, tag=f"c_{nm}")
                nc.sync.dma_start(tile[:], t[:])
                cs[nm] = tile

            ident = cp.tile([128, 128], F32, tag="ident")
            make_identity(nc, ident[:])

            xijT_all = cp.tile([128, tcn], F32, tag="xijT_all")

            def transpose_to(dst_ap, src_ap, eng="act"):
                tp = ps.tile([128, 128], F32, tag="p128")
                nc.tensor.transpose(tp[:], src_ap, ident[:])
                if eng == "act":
                    nc.scalar.copy(dst_ap, tp[:])
                else:
                    nc.vector.tensor_copy(out=dst_ap, in_=tp[:])

            # ---- PHASE A: per-edge features EC + xijT ------------------
            for j in range(ne if "a" in phases else 0):
                xi = wp.tile([128, C], F32, tag="xi")
                xj = wp.tile([128, C], F32, tag="xj")
                nc.gpsimd.indirect_dma_start(
                    out=xi[:], out_offset=None, in_=dt["x"][:],
                    in_offset=bass.IndirectOffsetOnAxis(
                        ap=cs["idx_t0"][:, j:j + 1], axis=0))
                nc.gpsimd.indirect_dma_start(
                    out=xj[:], out_offset=None, in_=dt["x"][:],
                    in_offset=bass.IndirectOffsetOnAxis(
                        ap=cs["idx_t1"][:, j:j + 1], axis=0))
                xij = wp.tile([128, C], F32, tag="xij")
                nc.vector.tensor_tensor(out=xij[:], in0=xi[:], in1=xj[:],
                                        op=ALU.mult)
                xiT = wp.tile([128, 128], F32, tag="xiT")
                xjT = wp.tile([128, 128], F32, tag="xjT")
                transpose_to(xiT[:], xi[:])
                transpose_to(xjT[:], xj[:])
                transpose_to(xijT_all[:, 128 * j:128 * (j + 1)], xij[:])

                ecp = ps.tile([128, 128], F32, tag="p128")
                nc.tensor.matmul(ecp[:], lhsT=xiT[:], rhs=cs["a1"][:],
                                 start=True, stop=False)
                nc.tensor.matmul(ecp[:], lhsT=xjT[:], rhs=cs["a2"][:],
                                 start=False, stop=False)
                nc.tensor.matmul(ecp[:], lhsT=xijT_all[:, 128 * j:128 * (j + 1)],
                                 rhs=cs["a3"][:], start=False, stop=False)
                nc.tensor.matmul(ecp[:], lhsT=cs["ones1"][:],
                                 rhs=cs["tokb_row"][:], start=False, stop=True)
                ec_s = wp.tile([128, 128], F32, tag="ec_s")
                nc.scalar.copy(ec_s[:], ecp[:])
                nc.sync.dma_start(ec_dram[128 * j:128 * (j + 1), :], ec_s[:])

            # ---- PHASE B: grouped, staged over 128-token tiles ---------
            # Stages per group of G tiles so ACT table funcs (sqrt/exp/gelu)
            # batch together: 4 table loads per G tiles instead of 4/tile.
            SHUF16 = [16] * 32
            GRP = 16
            nb = nt if "b" in phases else 0

            def s1a(m):
                """gather -> tok (relu'd) + LN1 stats; returns (tok, mv)."""
                xw = wp.tile([128, C], F32, tag="xw", name="xw")
                nc.gpsimd.indirect_dma_start(
                    out=xw[:], out_offset=None, in_=dt["x"][:],
                    in_offset=bass.IndirectOffsetOnAxis(
                        ap=cs["idx_cn"][:, m:m + 1], axis=0))
                ec4 = wp.tile([4, 128], F32, tag="ec4", name="ec4")
                nc.sync.dma_start(ec4[:], ec_dram[4 * m:4 * m + 4, :])
                xwT = wp.tile([128, 128], F32, tag="xwT", name="xwT")
                transpose_to(xwT[:], xw[:], eng="dve")
                tokp = ps.tile([128, 128], F32, tag="p128", name="tokp")
                nc.tensor.matmul(tokp[:], lhsT=xwT[:], rhs=cs["w0xT"][:],
                                 start=True, stop=False)
                nc.tensor.matmul(tokp[:], lhsT=cs["sel4"][:], rhs=ec4[:],
                                 start=False, stop=True)
                tok = cr.tile([128, 128], F32, tag="tok", name="tok")
                nc.scalar.activation(tok[:], tokp[:], AF.Relu)
                st = wp.tile([128, 6], F32, tag="ln_st", name="st")
                nc.vector.bn_stats(st[:], tok[:])
                mv = cr.tile([128, 2], F32, tag="mv", name="mv")
                nc.vector.bn_aggr(mv[:], st[:])
                return tok, mv

            def sqrt_of(mv, tag):
                std = cr.tile([128, 1], F32, tag=tag, name="std")
                nc.scalar.activation(std[:], mv[:, 1:2], AF.Sqrt,
                                     bias=cs["eps_col"][:, 0:1])
                return std

            def ln_apply(x, mv, std, tag):
                rstd = wp.tile([128, 1], F32, tag="rstd_" + tag, name="rstd")
                nc.vector.reciprocal(rstd[:], std[:])
                z = wp.tile([128, 128], F32, tag="z_" + tag, name="z")
                nc.vector.tensor_scalar(out=z[:], in0=x[:],
                                        scalar1=mv[:, 0:1],
                                        scalar2=rstd[:, 0:1],
                                        op0=ALU.subtract, op1=ALU.mult)
                return z

            def s1b_1(m, tok, mv, std):
                z1 = ln_apply(tok, mv, std, "1")
                z1T = wp.tile([128, 128], F32, tag="z1T", name="z1T", bufs=5)
                transpose_to(z1T[:], z1[:])

                kp = ps.tile([128, 128], F32, tag="p128", name="kp")
                nc.tensor.matmul(kp[:], lhsT=cs["wk_l"][:], rhs=z1T[:],
                                 start=True, stop=True)
                kTs = wp.tile([128, 128], F32, tag="kTs", name="kTs", bufs=5)
                nc.scalar.copy(kTs[:], kp[:])
                qbds = wp.tile([128, 1024], F32, tag="qbds", name="qbds",
                               bufs=5)
                for half in range(2):
                    qbdp = psb.tile([128, 512], F32, tag="big", name="qbdp")
                    for hh in range(4):
                        h = 4 * half + hh
                        nc.tensor.matmul(qbdp[:, 128 * hh:128 * (hh + 1)],
                                         lhsT=cs[f"wqbd_{h}"][:], rhs=z1T[:],
                                         start=True, stop=True)
                    if half == 0:
                        nc.vector.tensor_copy(out=qbds[:, 0:512], in_=qbdp[:])
                    else:
                        nc.scalar.copy(qbds[:, 512:1024], qbdp[:])
                vp = ps.tile([128, 129], F32, tag="p128", name="vp")
                nc.tensor.matmul(vp[:], lhsT=z1T[:], rhs=cs["wv_aug"][:],
                                 start=True, stop=True)
                v_s = wp.tile([128, 129], F32, tag="v_s", name="v_s", bufs=5)
                nc.scalar.copy(v_s[:], vp[:])
                ebias = wp.tile([128, 4], F32, tag="ebias", name="ebias",
                                bufs=5)
                nc.gpsimd.tensor_tensor(
                    out=ebias[:], in0=cs["vmbd"][:, 4 * m:4 * m + 4],
                    in1=v_s[:, 128:129].to_broadcast([128, 4]), op=ALU.add)
                vaug = wp.tile([128, 8 * 17], F32, tag="vaug", name="vaug",
                               bufs=5)
                va = vaug[:].rearrange("p (h d) -> p h d", d=17)
                nc.gpsimd.tensor_copy(
                    out=va[:, :, 0:16],
                    in_=v_s[:, 0:128].rearrange("p (h d) -> p h d", d=16))
                nc.gpsimd.tensor_copy(
                    out=va[:, :, 16:17],
                    in_=cs["valid"][:, m:m + 1].to_broadcast([128, 8, 1]))
                return kTs, qbds, ebias, vaug

            def s1b_2(m, kTs, qbds, ebias):
                E = wp.tile([128, 1024], F32, tag="E", name="E", bufs=5)
                for half in range(2):
                    sp = psb.tile([128, 512], F32, tag="big", name="sp")
                    nc.tensor.matmul(sp[:], lhsT=kTs[:],
                                     rhs=qbds[:, 512 * half:512 * (half + 1)],
                                     start=True, stop=True)
                    Ev = E[:, 512 * half:512 * (half + 1)].rearrange(
                        "p (h q) -> p h q", q=128)
                    sv = sp[:].rearrange("p (h q) -> p h q", q=128)
                    for e in range(4):
                        nc.scalar.activation(
                            Ev[:, :, 32 * e:32 * (e + 1)],
                            sv[:, :, 32 * e:32 * (e + 1)],
                            AF.Exp, bias=ebias[:, e:e + 1])
                return E

            def s1b_3(m, vaug, E):
                ctxp = psc.tile([128, 256], F32, tag="ctx", name="ctxp")
                for h in range(8):
                    co = 0 if h < 4 else 128
                    hh = 32 * (h % 4)
                    nc.tensor.matmul(
                        ctxp[hh:hh + 17, co:co + 128],
                        lhsT=vaug[:, 17 * h:17 * (h + 1)],
                        rhs=E[:, 128 * h:128 * (h + 1)],
                        start=True, stop=True, tile_position=(0, hh))
                cx = wp.tile([128, 256], F32, tag="cx", name="cx", bufs=5)
                nc.scalar.activation(cx[:], ctxp[:], AF.Identity,
                                     bias=cs["epsd_col"][:, 0:1])
                rt = wp.tile([128, 256], F32, tag="rt", name="rt")
                nc.vector.stream_shuffle(rt[:], cx[:], SHUF16)
                rtr = wp.tile([128, 256], F32, tag="rtr", name="rtr")
                nc.vector.reciprocal(rtr[:], rt[:])
                cn = wp.tile([128, 256], F32, tag="cn", name="cn", bufs=5)
                nc.vector.tensor_tensor(out=cn[:], in0=cx[:], in1=rtr[:],
                                        op=ALU.mult)
                return cn

            def s1b_4(m, tok, cn):
                up = ps.tile([128, 128], F32, tag="p128", name="up")
                nc.tensor.matmul(up[:], lhsT=cn[:, 0:128], rhs=cs["woutA"][:],
                                 start=True, stop=False)
                nc.tensor.matmul(up[:], lhsT=cn[:, 128:256],
                                 rhs=cs["woutB"][:], start=False, stop=False)
                nc.tensor.matmul(up[:], lhsT=cs["ones1"][:],
                                 rhs=cs["outb_row"][:], start=False, stop=True)
                tok2 = cr.tile([128, 128], F32, tag="tok2", name="tok2")
                nc.vector.tensor_tensor(out=tok2[:], in0=tok[:], in1=up[:],
                                        op=ALU.add)
                st2 = wp.tile([128, 6], F32, tag="ln_st2", name="st2")
                nc.vector.bn_stats(st2[:], tok2[:])
                mv2 = cr.tile([128, 2], F32, tag="mv2", name="mv2")
                nc.vector.bn_aggr(mv2[:], st2[:])
                return tok2, mv2

            def s1b_group(ms, d1, stds):
                SG = 4
                out = {}
                for i0 in range(0, len(ms), SG):
                    sub = ms[i0:i0 + SG]
                    st1 = {m: s1b_1(m, d1[m][0], d1[m][1], stds[m])
                           for m in sub}
                    eE = {m: s1b_2(m, st1[m][0], st1[m][1], st1[m][2])
                          for m in sub}
                    cns = {m: s1b_3(m, st1[m][3], eE[m]) for m in sub}
                    for m in sub:
                        out[m] = s1b_4(m, d1[m][0], cns[m])
                return out

            def s2(m, tok2, mv2, std2):
                """LN2 apply + ff + residual + pool."""
                z2 = ln_apply(tok2, mv2, std2, "2")
                z2T = wp.tile([128, 128], F32, tag="z2T", name="z2T")
                transpose_to(z2T[:], z2[:])
                gT = wp.tile([128, 512], F32, tag="gT", name="gT")
                fp = psb.tile([128, 512], F32, tag="big", name="fp")
                for c4 in range(4):
                    nc.tensor.matmul(fp[:, 128 * c4:128 * (c4 + 1)],
                                     lhsT=cs[f"wff1_{c4}"][:],
                                     rhs=z2T[:], start=True, stop=True)
                    nc.scalar.activation(gT[:, 128 * c4:128 * (c4 + 1)],
                                         fp[:, 128 * c4:128 * (c4 + 1)],
                                         AF.Gelu,
                                         bias=cs[f"bff1_{c4}"][:, 0:1])
                f2p = ps.tile([128, 128], F32, tag="p128", name="f2p")
                for c4 in range(4):
                    nc.tensor.matmul(f2p[:],
                                     lhsT=gT[:, 128 * c4:128 * (c4 + 1)],
                                     rhs=cs[f"wff2_{c4}"][:],
                                     start=(c4 == 0), stop=(c4 == 3))
                tok3 = wp.tile([128, 128], F32, tag="tok3", name="tok3")
                nc.vector.tensor_tensor(out=tok3[:], in0=tok2[:], in1=f2p[:],
                                        op=ALU.add)
                pp = ps.tile([4, 128], F32, tag="p128", name="pp")
                nc.tensor.matmul(pp[:], lhsT=cs["ebd"][:, 4 * m:4 * m + 4],
                                 rhs=tok3[:], start=True, stop=False)
                nc.tensor.matmul(pp[:], lhsT=cs["ind"][0:1, 4 * m:4 * m + 4],
                                 rhs=cs["bff2_row"][:], start=False, stop=True)
                pxs = wp.tile([4, 128], F32, tag="pxs", name="pxs")
                nc.scalar.copy(pxs[:], pp[:])
                nc.sync.dma_start(xcn_dram[4 * m:4 * m + 4, :], pxs[:])

            for g0 in range(0, nb, GRP):
                gms = list(range(g0, min(g0 + GRP, nb)))
                d1 = {m: s1a(m) for m in gms}
                stds = {m: sqrt_of(d1[m][1], "std1") for m in gms}
                d2 = s1b_group(gms, d1, stds)
                stds2 = {m: sqrt_of(d2[m][1], "std2") for m in gms}
                for m in gms:
                    s2(m, d2[m][0], d2[m][1], stds2[m])

            # ---- PHASE C ------------------------------------
            def _phase_c(lo, w):
                # ---- PHASE C: edge MLPs (edges [lo, lo+w)) -----------------
                xcnT = mp.tile([128, w], F32, tag="xcnT", name="xcnT")
                for j in range(lo // 128, (lo + w) // 128):
                    xct = wp.tile([128, 128], F32, tag="xct", name="xct")
                    nc.sync.dma_start(xct[:], xcn_dram[128 * j:128 * (j + 1), :])
                    transpose_to(xcnT[:, 128 * j - lo:128 * (j + 1) - lo],
                                 xct[:])

                def dense(rhs_tile, win, bin_, act, n_ic, out_tag):
                    """out[oc-chunk][128, w] = act(W @ rhs + b)."""
                    outs = []
                    for oc in range(2):
                        o = mp.tile([128, w], F32, tag=f"{out_tag}{oc}",
                                    name=out_tag)
                        for nh in range(max(1, w // 512)):
                            cw = min(512, w)
                            p5 = psb.tile([128, 512], F32, tag="big")
                            for ic in range(n_ic):
                                wt = cs[win(ic, oc)]
                                r = (rhs_tile if n_ic == 1 else rhs_tile[ic])
                                nc.tensor.matmul(
                                    p5[:, :cw], lhsT=wt[:],
                                    rhs=r[:, cw * nh:cw * (nh + 1)],
                                    start=(ic == 0), stop=(ic == n_ic - 1))
                            nc.scalar.activation(
                                o[:, cw * nh:cw * (nh + 1)], p5[:, :cw], act,
                                bias=cs[bin_(oc)][:, 0:1])
                        outs.append(o)
                    return outs

                h1 = dense(xcnT, lambda ic, oc: f"wx1_{oc}",
                           lambda oc: f"bx1_{oc}", AF.Relu, 1, "h1_")
                h2 = dense(h1, lambda ic, oc: f"wx2_{ic}{oc}",
                           lambda oc: f"bx2_{oc}", AF.Relu, 2, "h2_")
                h3 = dense(h2, lambda ic, oc: f"wx3_{ic}{oc}",
                           lambda oc: f"bx3_{oc}", AF.Identity, 2, "h3_")
                j1 = dense(xijT_all[:, lo:lo + w], lambda ic, oc: f"wxj1_{oc}",
                           lambda oc: f"bxj1_{oc}", AF.Relu, 1, "j1_")
                j2 = dense(j1, lambda ic, oc: f"wxj2_{ic}{oc}",
                           lambda oc: f"bxj2_{oc}", AF.Identity, 2, "j2_")
                zi = []
                for oc in range(2):
                    z = mp.tile([128, w], F32, tag=f"zi{oc}", name="zi")
                    nc.vector.scalar_tensor_tensor(
                        out=z[:], in0=h3[oc][:], scalar=cs["beta_col"][:, 0:1],
                        in1=j2[oc][:], op0=ALU.mult, op1=ALU.add)
                    zi.append(z)
                zz = dense(zi, lambda ic, oc: f"wl1_{ic}{oc}",
                           lambda oc: f"bl1_{oc}", AF.Relu, 2, "zz")

                osb = mp.tile([1, w], F32, tag="osb", name="osb")
                cw = min(512, w)
                for nh in range(max(1, w // 512)):
                    fo = ps.tile([1, 512], F32, tag="p128")
                    nc.tensor.matmul(fo[:, :cw], lhsT=cs["wl2_0"][:],
                                     rhs=zz[0][:, cw * nh:cw * (nh + 1)],
                                     start=True, stop=False)
                    nc.tensor.matmul(fo[:, :cw], lhsT=cs["wl2_1"][:],
                                     rhs=zz[1][:, cw * nh:cw * (nh + 1)],
                                     start=False, stop=True)
                    nc.scalar.activation(osb[0:1, cw * nh:cw * (nh + 1)],
                                         fo[:, :cw],
                                         AF.Identity, bias=cs["bl2"][0:1, 0:1])
                nc.sync.dma_start(out_dram[0:1, lo:lo + w], osb[:])

            if "c" in phases:
                for _lo in range(0, tcn, 512):
                    _phase_c(_lo, min(512, tcn - _lo))
            else:
                dumm = mp.tile([1, tcn], F32, tag="dumm")
                nc.vector.memset(dumm[:], 0.0)
                nc.sync.dma_start(out_dram[:], dumm[:])

    nc.finalize()
    return nc


def _ln_stats(nc, wp, x, z_out, eps_col):
    """z = (x - mean(x)) / sqrt(var(x) + 1e-5) along the free dim."""
    st = wp.tile([128, 6], F32, tag="ln_st")
    nc.vector.bn_stats(st[:], x[:])
    mv = wp.tile([128, 2], F32, tag="ln_mv")
    nc.vector.bn_aggr(mv[:], st[:])
    std = wp.tile([128, 1], F32, tag="ln_std")
    nc.scalar.activation(std[:], mv[:, 1:2], AF.Sqrt, bias=eps_col[:, 0:1])
    rstd = wp.tile([128, 1], F32, tag="ln_rstd")
    nc.vector.reciprocal(rstd[:], std[:])
    nc.vector.tensor_scalar(out=z_out[:], in0=x[:], scalar1=mv[:, 0:1],
                            scalar2=rstd[:, 0:1], op0=ALU.subtract,
                            op1=ALU.mult)


# ---------------------------------------------------------------- host side

def _slot_pad_w(Weff, beff, heads):
    """[128c, 128slot] lhsT with 4 heads in 32-slots (16 data + 16 zero)."""
    w = np.zeros((128, 128), np.float32)
    b = np.zeros((128, 1), np.float32)
    for i, h in enumerate(heads):
        w[:, 32 * i:32 * i + 16] = Weff[16 * h:16 * h + 16, :].T
        b[32 * i:32 * i + 16, 0] = beff[16 * h:16 * h + 16]
    return w, b


def _prep_shared(inp):
    f = lambda k: np.asarray(inp[k], np.float32)
    tok_w, tok_b = f("tok_w"), f("tok_b")
    g1, b1 = f("ln1_g"), f("ln1_b")
    qkv_w, qkv_b = f("qkv_w"), f("qkv_b")
    out_w, out_b = f("out_w"), f("out_b")
    g2, b2 = f("ln2_g"), f("ln2_b")
    ff1_w, ff1_b = f("ff1_w"), f("ff1_b")
    ff2_w, ff2_b = f("ff2_w"), f("ff2_b")

    d = {}
    d["w0xT"] = tok_w[:, :C].T.copy()
    d["a1"] = tok_w[:, C:2 * C].T.copy()
    d["a2"] = tok_w[:, 2 * C:3 * C].T.copy()
    d["a3"] = tok_w[:, 3 * C:4 * C].T.copy()
    d["tokb_row"] = tok_b[None, :].copy()

    sc = 1.0 / np.sqrt(DH)
    Wq, Wk, Wv = qkv_w[:C], qkv_w[C:2 * C], qkv_w[2 * C:3 * C]
    bq, bk, bv = qkv_b[:C], qkv_b[C:2 * C], qkv_b[2 * C:3 * C]
    Wq_e = Wq * g1[None, :] * sc
    bq_e = (Wq @ b1) * sc + bq * sc
    Wk_e = Wk * g1[None, :]
    Wv_e = Wv * g1[None, :]
    bv_e = Wv @ b1 + bv
    d["wk_l"] = Wk_e.T.copy()
    for h in range(8):
        w = np.zeros((128, 128), np.float32)
        rows = slice(16 * h, 16 * (h + 1))
        w[:, rows] = Wq_e[rows, :].T
        d[f"wqbd_{h}"] = w
    # q/k additive biases: per-qtok terms cancel in softmax; the per-ktok
    # term c_k = bq_e . k(token) is linear in z1 -> extra v output channel.
    w_ck = Wk_e.T @ bq_e                      # [128 in-c]
    d["wv_aug"] = np.concatenate([Wv_e.T, w_ck[:, None]], axis=1).copy()

    for nm, heads in (("woutA", [0, 1, 2, 3]), ("woutB", [4, 5, 6, 7])):
        w = np.zeros((128, 128), np.float32)
        for i, h in enumerate(heads):
            w[32 * i:32 * i + 16, :] = out_w[:, 16 * h:16 * h + 16].T
        d[nm] = w
    d["outb_row"] = (out_b + out_w @ bv_e)[None, :].copy()

    for c4 in range(4):
        sl = slice(128 * c4, 128 * (c4 + 1))
        d[f"wff1_{c4}"] = (ff1_w[sl, :] * g2[None, :]).T.copy()
        d[f"bff1_{c4}"] = (ff1_w[sl, :] @ b2 + ff1_b[sl])[:, None].copy()
        d[f"wff2_{c4}"] = ff2_w[:, sl].T.copy()
    d["bff2_row"] = ff2_b[None, :].copy()

    for nm, wkey, bkey in (("wx1", "xcn_w1", "xcn_b1"),
                           ("wxj1", "xij_w1", "xij_b1")):
        W, B = f(wkey), f(bkey)
        for oc in range(2):
            sl = slice(128 * oc, 128 * (oc + 1))
            d[f"{nm}_{oc}"] = W[sl, :].T.copy()
            d[f"b{nm[1:]}_{oc}"] = B[sl][:, None].copy()
    for nm, wkey, bkey in (("wx2", "xcn_w2", "xcn_b2"),
                           ("wx3", "xcn_w3", "xcn_b3"),
                           ("wxj2", "xij_w2", "xij_b2"),
                           ("wl1", "lin_w1", "lin_b1")):
        W, B = f(wkey), f(bkey)
        for ic in range(2):
            for oc in range(2):
                d[f"{nm}_{ic}{oc}"] = \
                    W[128 * oc:128 * (oc + 1), 128 * ic:128 * (ic + 1)].T.copy()
        for oc in range(2):
            d[f"b{nm[1:]}_{oc}"] = B[128 * oc:128 * (oc + 1)][:, None].copy()
    lin_w2, lin_b2 = f("lin_w2"), f("lin_b2")
    d["wl2_0"] = lin_w2[0, :128][:, None].copy()
    d["wl2_1"] = lin_w2[0, 128:][:, None].copy()
    d["bl2"] = lin_b2.reshape(1, 1).copy()

    sel4 = np.zeros((4, 128), np.float32)
    for e in range(4):
        sel4[e, 32 * e:32 * (e + 1)] = 1.0
    d["sel4"] = sel4
    d["ones1"] = np.ones((1, 128), np.float32)
    d["eps_col"] = np.full((128, 1), 1e-5, np.float32)
    d["epsd_col"] = np.full((128, 1), 1e-30, np.float32)
    d["beta_col"] = np.full((128, 1), np.asarray(inp["beta"],
                                                 np.float32).reshape(-1)[0])
    return {k: np.ascontiguousarray(v, np.float32) for k, v in d.items()}


def _prep_core(inp, core, nt=NT):
    ne = 4 * nt // 128
    sl = slice(core * TC, (core + 1) * TC)
    tar = np.asarray(inp["tar_ei"])[:, sl].astype(np.int32)
    cols = np.asarray(inp["cn_cols"])[sl].astype(np.int32)     # [TC, K]
    cnt = np.asarray(inp["cn_counts"])[sl].astype(np.int64)    # [TC]

    d = {}
    d["idx_cn"] = np.ascontiguousarray(cols.reshape(-1).reshape(NT, 128).T)[:, :nt]
    d["idx_t0"] = np.ascontiguousarray(tar[0].reshape(NE, 128).T)[:, :ne]
    d["idx_t1"] = np.ascontiguousarray(tar[1].reshape(NE, 128).T)[:, :ne]

    kk = np.arange(K)
    valid_ek = (kk[None, :] < cnt[:, None])                    # [TC, K] bool
    valid_flat = valid_ek.reshape(-1)                          # token-order
    d["valid"] = np.ascontiguousarray(
        valid_flat.reshape(NT, 128).T.astype(np.float32))

    p = np.arange(128)
    pe = p // 32                                               # edge slot of row
    vmbd = np.zeros((128, 4 * NT), np.float32)
    ebd = np.zeros((128, 4 * NT), np.float32)
    vf = d["valid"]                                            # [128, NT]
    rc = (1.0 / np.maximum(cnt, 1)).astype(np.float32)         # [TC]
    for e in range(4):
        onblk = (pe == e)                                      # [128]
        vmbd[:, e::4] = NEG * (~(onblk[:, None] & (vf > 0))).astype(np.float32)
        ebd[:, e::4] = (onblk[:, None] * vf) * rc.reshape(NT, 4).T[e][None, :]
    d["vmbd"] = vmbd[:, :4 * nt]
    d["ebd"] = ebd[:, :4 * nt]
    d["ind"] = (cnt > 0).astype(np.float32)[None, :4 * nt].copy()
    d["valid"] = d["valid"][:, :nt].copy()
    return {k: np.ascontiguousarray(v) for k, v in d.items()}


_CACHE = {}
_CACHE_LOCK = threading.Lock()


def _get_nc(nt=NT, phases="abc", bsteps=99):
    with _CACHE_LOCK:
        key = (nt, phases, bsteps)
        if key not in _CACHE:
            _CACHE[key] = _build_nc(nt, phases, bsteps)
        return _CACHE[key]


def run(inputs, nt=NT, phases="abc", bsteps=99, **spmd_kwargs):
    """Run the kernel on the first 4*nt edges of each core's shard.

    Returns (out [NCORES, 4*nt], BassKernelResults).
    """
    nc = _get_nc(nt, phases, bsteps)
    shared = _prep_shared(inputs)
    x = np.ascontiguousarray(np.asarray(inputs["x"], np.float32))
    in_maps = []
    for core in range(NCORES):
        m = dict(shared)
        m["x"] = x
        m.update(_prep_core(inputs, core, nt))
        in_maps.append(m)
    res = run_bass_kernel_spmd(nc, in_maps, core_ids=list(range(NCORES)),
                               **spmd_kwargs)
    out = np.stack([res.results[c]["out"][0] for c in range(NCORES)])
    return out, res


def kernel(**inputs):
    out, _ = run(inputs)
    return out.reshape(T, O).astype(np.float32)



# revision 14
# speedup vs baseline: 2.2521x; 2.2521x over previous
"""Trainium2 Bass kernel for nn_CNLinkPredictor (gnn_message_passing), v2.

Data-parallel over target edges T (8192) across 8 NeuronCores (1024
edges/core).  v2 reworks the baseline around three findings from the HW
profile: fp32 matmuls run as 2 half-rate passes (4 cyc/row vs 1 for bf16),
ACT/DVE per-instruction overheads demand 512-wide ops, and ACT table
switches cost 1.3us.

Layout: channel-major [128 ch, 512 tok] supertiles (16 edges each);
 - all matmuls bf16 (PSUM f32); x is pre-cast to bf16 in DRAM.
 - LN has no affine (folded into following weights); stats are computed
   with per-token-column outputs via tok-chunk-stationary matmuls, the
   scalar math runs on [128,4] tiles, and rstd/-mu*rstd rows are
   transposed once and broadcast with rank-1 ones matmuls.
 - mean-subtraction for the q/k/v projections is folded into rank-1
   PSUM corrections (W@1 outer -mu*rstd), so z1 is just tok*rstd_bc.
 - per-key softmax bias exp(c_k) and key-validity are folded into a
   post-exp scaling eps of V (and the denominator aug channel); the
   cross-edge block mask is a constant bf16 0/1 multiply on E.
 - scores use a band-structured q ("qbds", built with SBUF->SBUF DMAs,
   zeros persistent) against dense channel-major k slices.
 - pooling = masked (valid/cnt) multiply + segmented DVE reduce.
 - phases grouped G=8 supertiles so ACT tables (sqrt/exp/gelu) load
   once per group.
"""

import sys
import threading

sys.path.insert(0, "/opt/trn_rl_repo")

import numpy as np
import ml_dtypes

import concourse.bass as bass
import concourse.bacc as bacc
import concourse.mybir as mybir
from concourse.tile import TileContext
from concourse.masks import make_identity
from concourse.bass_utils import run_bass_kernel_spmd

F32 = mybir.dt.float32
BF16 = mybir.dt.bfloat16
I32 = mybir.dt.int32
AF = mybir.ActivationFunctionType
ALU = mybir.AluOpType

N, C, H, O = 100000, 128, 256, 1
T, K = 8192, 32
NHEAD, DH, FF = 8, 16, 512
NCORES = 8
TC = T // NCORES          # 1024 edges per core
NST = TC // 16            # 64 supertiles (512 tokens / 16 edges each)
GRP = 8                   # supertiles per table-phase group
NEG = -1e9

bfa = lambda a: np.ascontiguousarray(np.asarray(a, np.float32)).astype(ml_dtypes.bfloat16)
f32a = lambda a: np.ascontiguousarray(np.asarray(a, np.float32))


def _build_nc(nst=NST):
    nc = bacc.Bacc("TRN2", target_bir_lowering=False)
    tcn = 16 * nst                     # edges this build covers per core
    ne = TC // 128                     # 8 phase-A tiles (always full)

    dt = {}

    def din(name, shape, dtype=BF16):
        dt[name] = nc.dram_tensor(name, shape, dtype, kind="ExternalInput")
        return dt[name]

    # data
    din("xbf", [N, C])
    din("idx_cn", [128, 4 * NST], I32)
    din("idx_t0", [128, ne], I32)
    din("idx_t1", [128, ne], I32)
    din("negv", [128, 4 * NST], F32)
    din("msbig", [128, 512 * NST])
    # weights
    for nm in ["w0xT", "a1", "a2", "a3", "wqT", "wkT", "woutA", "woutB",
               "wf1T_0", "wf1T_1", "wf1T_2", "wf1T_3",
               "wf2T_0", "wf2T_1", "wf2T_2", "wf2T_3",
               "wx1_0", "wx1_1", "wxj1_0", "wxj1_1"]:
        din(nm, [128, 128])
    for ic in range(2):
        for oc in range(2):
            for nm in ["wx2", "wx3", "wxj2", "wl1"]:
                din(f"{nm}_{ic}{oc}", [128, 128])
    din("wv_aug", [128, 129])
    din("wl2_0", [128, 1])
    din("wl2_1", [128, 1])
    din("Bind", [16, 512])
    din("bandmask", [128, 1024])
    din("wmean", [128, 1])
    din("ones_rep", [128, 128])
    din("tokb_row", [1, 128])
    din("wq1_rep", [128, 128])
    din("wk1_rep", [128, 128])
    din("wv1_rep", [128, 129])
    for nm in ["outb_col", "bff2_col", "eps_col", "epsd_col", "beta_col",
               "bx1_0", "bx1_1", "bx2_0", "bx2_1", "bx3_0", "bx3_1",
               "bxj1_0", "bxj1_1", "bxj2_0", "bxj2_1", "bl1_0", "bl1_1"]:
        din(nm, [128, 1], F32)
    for c4 in range(4):
        din(f"bff1_{c4}", [128, 1], F32)
    din("bl2", [1, 1], F32)

    ec_dram = nc.dram_tensor("ec_dram", [TC, 128], BF16)
    out_dram = nc.dram_tensor("out", [1, tcn], F32, kind="ExternalOutput")

    with TileContext(nc) as tc:
        with (
            nc.allow_low_precision(reason="bf16 pipeline validated vs ref"),
            tc.tile_pool(name="cpool", bufs=1) as cp,
            tc.tile_pool(name="wp", bufs=2) as wp,
            tc.tile_pool(name="mp", bufs=2) as mp,
            tc.tile_pool(name="p1024", bufs=2, space="PSUM") as p1024,
            tc.tile_pool(name="pacc", bufs=1, space="PSUM") as pacc,
            tc.tile_pool(name="pT", bufs=1, space="PSUM") as pTp,
            tc.tile_pool(name="pctx", bufs=2, space="PSUM") as pctxp,
        ):
            cs = {}
            for nm, t in dt.items():
                if nm in ("xbf", "msbig"):
                    continue
                tile = cp.tile(list(t.shape), t.dtype, tag=f"c_{nm}", name=nm)
                nc.sync.dma_start(tile[:], t[:])
                cs[nm] = tile

            ident = cp.tile([128, 128], BF16, tag="ident")
            make_identity(nc, ident[:])

            xijT_all = cp.tile([128, TC], BF16, tag="xijT_all")
            xcn_all = cp.tile([128, tcn], F32, tag="xcn_all")
            qbds = cp.tile([128, 4096], BF16, tag="qbds")
            nc.vector.memset(qbds[:], 0.0)

            # ---------------- PHASE A: per-edge EC + xijT ----------------
            for j in range(ne):
                xi = wp.tile([128, C], BF16, tag="xi")
                xj = wp.tile([128, C], BF16, tag="xj")
                nc.gpsimd.indirect_dma_start(
                    out=xi[:], out_offset=None, in_=dt["xbf"][:],
                    in_offset=bass.IndirectOffsetOnAxis(
                        ap=cs["idx_t0"][:, j:j + 1], axis=0))
                nc.gpsimd.indirect_dma_start(
                    out=xj[:], out_offset=None, in_=dt["xbf"][:],
                    in_offset=bass.IndirectOffsetOnAxis(
                        ap=cs["idx_t1"][:, j:j + 1], axis=0))
                xij = wp.tile([128, C], BF16, tag="xij")
                nc.vector.tensor_tensor(out=xij[:], in0=xi[:], in1=xj[:],
                                        op=ALU.mult)
                pt = pTp.tile([128, 512], BF16, tag="pT", name="pt")
                nc.tensor.transpose(pt[:, 0:128], xi[:], ident[:])
                nc.tensor.transpose(pt[:, 128:256], xj[:], ident[:])
                nc.tensor.transpose(pt[:, 256:384], xij[:], ident[:])
                xiT = wp.tile([128, 128], BF16, tag="xiT")
                nc.vector.tensor_copy(out=xiT[:], in_=pt[:, 0:128])
                xjT = wp.tile([128, 128], BF16, tag="xjT")
                nc.vector.tensor_copy(out=xjT[:], in_=pt[:, 128:256])
                nc.vector.tensor_copy(out=xijT_all[:, 128 * j:128 * (j + 1)],
                                      in_=pt[:, 256:384])

                ecp = pctxp.tile([128, 258], F32, tag="pctx", name="ecp")
                nc.tensor.matmul(ecp[:, 0:128], lhsT=xiT[:], rhs=cs["a1"][:],
                                 start=True, stop=False)
                nc.tensor.matmul(ecp[:, 0:128], lhsT=xjT[:], rhs=cs["a2"][:],
                                 start=False, stop=False)
                nc.tensor.matmul(ecp[:, 0:128],
                                 lhsT=xijT_all[:, 128 * j:128 * (j + 1)],
                                 rhs=cs["a3"][:], start=False, stop=False)
                nc.tensor.matmul(ecp[:, 0:128], lhsT=cs["ones_rep"][0:1, :],
                                 rhs=cs["tokb_row"][:], start=False, stop=True)
                ec_s = wp.tile([128, 128], BF16, tag="ec_s")
                nc.vector.tensor_copy(out=ec_s[:], in_=ecp[:, 0:128])
                nc.sync.dma_start(ec_dram[128 * j:128 * (j + 1), :], ec_s[:])

            # ---------------- PHASE B: grouped supertiles ----------------
            SHUF16 = [16] * 32

            def s1(g):
                """gather + transpose + tok(relu) + LN1 raw stats."""
                xw = wp.tile([128, 512], BF16, tag="xw", name="xw")
                for s in range(4):
                    nc.gpsimd.indirect_dma_start(
                        out=xw[:, 128 * s:128 * (s + 1)], out_offset=None,
                        in_=dt["xbf"][:],
                        in_offset=bass.IndirectOffsetOnAxis(
                            ap=cs["idx_cn"][:, 4 * g + s:4 * g + s + 1],
                            axis=0))
                pt = pTp.tile([128, 512], BF16, tag="pT", name="pt")
                for s in range(4):
                    nc.tensor.transpose(pt[:, 128 * s:128 * (s + 1)],
                                        xw[:, 128 * s:128 * (s + 1)], ident[:])
                xwcm = wp.tile([128, 512], BF16, tag="xwcm", name="xwcm")
                nc.vector.tensor_copy(out=xwcm[:], in_=pt[:])
                ec16 = wp.tile([16, 128], BF16, tag="ec16", name="ec16")
                nc.sync.dma_start(ec16[:], ec_dram[16 * g:16 * (g + 1), :])
                tokp = pacc.tile([128, 512], F32, tag="pacc", name="tokp")
                nc.tensor.matmul(tokp[:], lhsT=cs["w0xT"][:], rhs=xwcm[:],
                                 start=True, stop=False)
                nc.tensor.matmul(tokp[:], lhsT=ec16[:], rhs=cs["Bind"][:],
                                 start=False, stop=True)
                tok = wp.tile([128, 512], BF16, tag="tok", name="tok", bufs=9)
                nc.scalar.activation(tok[:], tokp[:], AF.Relu)
                sq = wp.tile([128, 512], BF16, tag="sq", name="sq")
                nc.vector.tensor_tensor(out=sq[:], in0=tok[:], in1=tok[:],
                                        op=ALU.mult)
                stp = pctxp.tile([128, 258], F32, tag="pctx", name="stp")
                for s in range(4):
                    nc.tensor.matmul(stp[:, s:s + 1],
                                     lhsT=tok[:, 128 * s:128 * (s + 1)],
                                     rhs=cs["wmean"][:], start=True, stop=True)
                    nc.tensor.matmul(stp[:, 4 + s:5 + s],
                                     lhsT=sq[:, 128 * s:128 * (s + 1)],
                                     rhs=cs["wmean"][:], start=True, stop=True)
                st_sb = wp.tile([128, 8], F32, tag="st_sb", name="st_sb",
                                bufs=9)
                nc.vector.tensor_copy(out=st_sb[:], in_=stp[:, 0:8])
                return tok, st_sb

            def s2(g, st_sb, tag):
                """[128,4] scalar math -> rowT [8,128] = {rstd | -mu*rstd}."""
                mu = st_sb[:, 0:4]
                rows = wp.tile([128, 8], F32, tag="rows" + tag, name="rows")
                musq = wp.tile([128, 4], F32, tag="musq" + tag, name="musq")
                nc.vector.tensor_tensor(out=musq[:], in0=mu, in1=mu,
                                        op=ALU.mult)
                varr = wp.tile([128, 4], F32, tag="varr" + tag, name="varr")
                nc.vector.tensor_tensor(out=varr[:], in0=st_sb[:, 4:8],
                                        in1=musq[:], op=ALU.subtract)
                stdd = wp.tile([128, 4], F32, tag="stdd" + tag, name="stdd")
                nc.scalar.activation(stdd[:], varr[:], AF.Sqrt,
                                     bias=cs["eps_col"][:, 0:1])
                nc.vector.reciprocal(rows[:, 0:4], stdd[:])
                negmu = wp.tile([128, 4], F32, tag="negmu" + tag, name="negmu")
                nc.vector.tensor_scalar(out=negmu[:], in0=mu, scalar1=-1.0,
                                        scalar2=None, op0=ALU.mult)
                nc.vector.tensor_tensor(out=rows[:, 4:8], in0=negmu[:],
                                        in1=rows[:, 0:4], op=ALU.mult)
                # place the 8 per-subtile scalars into columns whose
                # transposed rows land on legal PE base partitions (0/32/64):
                # s<3: rowT[32s, 0:128]=rstd_s, rowT[32s, 128:256]=-mu*rstd_s
                # s=3: rowT[0, 256:384]=rstd_3, rowT[0, 384:512]=-mu*rstd_3
                rsp = wp.tile([128, 512], BF16, tag="rsp" + tag, name="rsp")
                nc.vector.tensor_copy(
                    out=rsp[:, 0:96].rearrange("p (s o) -> p s o", o=32)[:, :, 0:1],
                    in_=rows[:, 0:3].rearrange("p (s o) -> p s o", o=1))
                nc.vector.tensor_copy(
                    out=rsp[:, 128:224].rearrange("p (s o) -> p s o", o=32)[:, :, 0:1],
                    in_=rows[:, 4:7].rearrange("p (s o) -> p s o", o=1))
                nc.vector.tensor_copy(out=rsp[:, 256:257], in_=rows[:, 3:4])
                nc.vector.tensor_copy(out=rsp[:, 384:385], in_=rows[:, 7:8])
                pt = pTp.tile([128, 512], BF16, tag="pT", name="pt")
                for c in range(4):
                    nc.tensor.transpose(pt[:, 128 * c:128 * (c + 1)],
                                        rsp[:, 128 * c:128 * (c + 1)],
                                        ident[:])
                rts = []
                for s in range(4):
                    rt0 = wp.tile([1, 256], BF16, tag=f"rowT{tag}_{s}",
                                  name="rt0", bufs=9)
                    if s < 3:
                        nc.vector.tensor_copy(out=rt0[:],
                                              in_=pt[32 * s:32 * s + 1, 0:256])
                    else:
                        nc.vector.tensor_copy(out=rt0[:],
                                              in_=pt[0:1, 256:512])
                    rts.append(rt0)
                return rts

            def rstd_ap(rts, s):
                return rts[s][0:1, 0:128]

            def negmu_ap(rts, s):
                return rts[s][0:1, 128:256]

            def base_of(s):
                return 0

            def s3(g, tok, rowT):
                """attention + out-proj + residual + LN2 raw stats."""
                # rstd broadcast [128, 512] via rank-1s
                zbc = p1024.tile([128, 1024], F32, tag="p1024", name="zbc")
                for s in range(4):
                    nc.tensor.matmul(zbc[:, 128 * s:128 * (s + 1)],
                                     lhsT=cs["ones_rep"][0:1, :],
                                     rhs=rstd_ap(rowT, s),
                                     start=True, stop=True)
                z1 = wp.tile([128, 512], BF16, tag="z1", name="z1")
                nc.vector.tensor_tensor(out=z1[:], in0=tok[:],
                                        in1=zbc[:, 0:512], op=ALU.mult)
                # q | k  (with rank-1 -mu*rstd corrections)
                qkp = p1024.tile([128, 1024], F32, tag="p1024", name="qkp")
                nc.tensor.matmul(qkp[:, 0:512], lhsT=cs["wqT"][:], rhs=z1[:],
                                 start=True, stop=False)
                for s in range(4):
                    nc.tensor.matmul(qkp[:, 128 * s:128 * (s + 1)],
                                     lhsT=cs["wq1_rep"][0:1, :],
                                     rhs=negmu_ap(rowT, s),
                                     start=False, stop=True)
                nc.tensor.matmul(qkp[:, 512:1024], lhsT=cs["wkT"][:],
                                 rhs=z1[:], start=True, stop=False)
                for s in range(4):
                    nc.tensor.matmul(qkp[:, 512 + 128 * s:512 + 128 * (s + 1)],
                                     lhsT=cs["wk1_rep"][0:1, :],
                                     rhs=negmu_ap(rowT, s),
                                     start=False, stop=True)
                qk = wp.tile([128, 1024], BF16, tag="qk", name="qk")
                nc.scalar.copy(qk[:, 0:512], qkp[:, 0:512])
                nc.scalar.copy(qk[:, 512:1024], qkp[:, 512:1024])
                # qbds bands via sbuf->sbuf DMA (zeros persistent)
                for h in range(8):
                    nc.sync.dma_start(
                        qbds[16 * h:16 * h + 16, :].rearrange(
                            "p (s hh q) -> p s hh q", hh=8, q=128)[:, :, h, :],
                        qk[16 * h:16 * h + 16, 0:512].rearrange(
                            "p (s q) -> p s q", q=128))
                # v + eps + vaug per subtile
                vaugs = []
                for p2 in range(2):
                    vp = pctxp.tile([128, 258], F32, tag="pctx", name="vp")
                    for i in range(2):
                        s = 2 * p2 + i
                        nc.tensor.matmul(vp[:, 129 * i:129 * i + 129],
                                         lhsT=z1[:, 128 * s:128 * (s + 1)],
                                         rhs=cs["wv_aug"][:],
                                         start=True, stop=False)
                        nc.tensor.matmul(vp[:, 129 * i:129 * i + 129],
                                         lhsT=negmu_ap(rowT, s),
                                         rhs=cs["wv1_rep"][0:1, :],
                                         start=False, stop=True)
                    for i in range(2):
                        s = 2 * p2 + i
                        v_sb = wp.tile([128, 129], BF16, tag="v_sb",
                                       name="v_sb", bufs=4)
                        nc.vector.tensor_copy(out=v_sb[:],
                                              in_=vp[:, 129 * i:129 * i + 129])
                        epsc = wp.tile([128, 1], F32, tag="epsc", name="epsc",
                                       bufs=4)
                        nc.scalar.activation(
                            epsc[:], v_sb[:, 128:129], AF.Exp,
                            bias=cs["negv"][:, 4 * g + s:4 * g + s + 1])
                        vaug = wp.tile([128, 136], BF16, tag="vaug",
                                       name="vaug", bufs=4)
                        vv = vaug[:].rearrange("p (h d) -> p h d", d=17)
                        nc.gpsimd.tensor_scalar(
                            out=vv[:, :, 0:16],
                            in0=v_sb[:, 0:128].rearrange("p (h d) -> p h d",
                                                         d=16),
                            scalar1=epsc[:, 0:1], scalar2=None, op0=ALU.mult)
                        nc.gpsimd.tensor_copy(
                            out=vv[:, :, 16:17],
                            in_=epsc[:, 0:1].to_broadcast([128, 8, 1]))
                        vaugs.append(vaug)
                # scores -> exp -> mask -> ctx -> norm, per subtile
                ups = pacc.tile([128, 512], F32, tag="pacc", name="ups")
                for s in range(4):
                    scp = p1024.tile([128, 1024], F32, tag="p1024", name="scp")
                    for half in range(2):
                        nc.tensor.matmul(
                            scp[:, 512 * half:512 * (half + 1)],
                            lhsT=qk[:, 512 + 128 * s:512 + 128 * (s + 1)],
                            rhs=qbds[:, 1024 * s + 512 * half:
                                     1024 * s + 512 * (half + 1)],
                            start=True, stop=True)
                    E = wp.tile([128, 1024], BF16, tag="E", name="E")
                    nc.scalar.activation(E[:, 0:512], scp[:, 0:512], AF.Exp)
                    nc.scalar.activation(E[:, 512:1024], scp[:, 512:1024],
                                         AF.Exp)
                    Em = wp.tile([128, 1024], BF16, tag="Em", name="Em")
                    nc.vector.tensor_tensor(out=Em[:], in0=E[:],
                                            in1=cs["bandmask"][:],
                                            op=ALU.mult)
                    ctxp = pctxp.tile([128, 258], F32, tag="pctx", name="ctxp")
                    for h in range(8):
                        co = 128 * (h // 4)
                        hh = 32 * (h % 4)
                        nc.tensor.matmul(
                            ctxp[hh:hh + 17, co:co + 128],
                            lhsT=vaugs[s][:, 17 * h:17 * h + 17],
                            rhs=Em[:, 128 * h:128 * (h + 1)],
                            start=True, stop=True, tile_position=(0, hh))
                    cx = wp.tile([128, 256], BF16, tag="cx", name="cx")
                    nc.vector.tensor_scalar(out=cx[:], in0=ctxp[:, 0:256],
                                            scalar1=cs["epsd_col"][:, 0:1],
                                            scalar2=None, op0=ALU.add)
                    rt = wp.tile([128, 256], BF16, tag="rt", name="rt")
                    nc.vector.stream_shuffle(rt[:], cx[:], SHUF16)
                    rtr = wp.tile([128, 256], BF16, tag="rtr", name="rtr")
                    nc.vector.reciprocal(rtr[:], rt[:])
                    cn = wp.tile([128, 256], BF16, tag="cn", name="cn")
                    nc.vector.tensor_tensor(out=cn[:], in0=cx[:], in1=rtr[:],
                                            op=ALU.mult)
                    nc.tensor.matmul(ups[:, 128 * s:128 * (s + 1)],
                                     lhsT=cs["woutA"][:], rhs=cn[:, 0:128],
                                     start=True, stop=False)
                    nc.tensor.matmul(ups[:, 128 * s:128 * (s + 1)],
                                     lhsT=cs["woutB"][:], rhs=cn[:, 128:256],
                                     start=False, stop=True)
                tok2 = wp.tile([128, 512], BF16, tag="tok2", name="tok2",
                               bufs=9)
                nc.vector.scalar_tensor_tensor(
                    out=tok2[:], in0=ups[:], scalar=cs["outb_col"][:, 0:1],
                    in1=tok[:], op0=ALU.add, op1=ALU.add)
                sq2 = wp.tile([128, 512], BF16, tag="sq2", name="sq2")
                nc.vector.tensor_tensor(out=sq2[:], in0=tok2[:], in1=tok2[:],
                                        op=ALU.mult)
                stp = pctxp.tile([128, 258], F32, tag="pctx", name="stp2")
                for s in range(4):
                    nc.tensor.matmul(stp[:, s:s + 1],
                                     lhsT=tok2[:, 128 * s:128 * (s + 1)],
                                     rhs=cs["wmean"][:], start=True, stop=True)
                    nc.tensor.matmul(stp[:, 4 + s:5 + s],
                                     lhsT=sq2[:, 128 * s:128 * (s + 1)],
                                     rhs=cs["wmean"][:], start=True, stop=True)
                st2_sb = wp.tile([128, 8], F32, tag="st2_sb", name="st2_sb",
                                 bufs=9)
                nc.vector.tensor_copy(out=st2_sb[:], in_=stp[:, 0:8])
                return tok2, st2_sb

            def s5(g, tok2, rowT2):
                """LN2 apply + FF + residual + masked pool -> xcn_all."""
                zbc = p1024.tile([128, 1024], F32, tag="p1024", name="zbc2")
                for i in range(8):
                    s, half = i % 4, i // 4
                    nc.tensor.matmul(zbc[:, 128 * i:128 * (i + 1)],
                                     lhsT=cs["ones_rep"][0:1, :],
                                     rhs=(rstd_ap(rowT2, s) if half == 0
                                          else negmu_ap(rowT2, s)),
                                     start=True, stop=True)
                z2t = wp.tile([128, 512], BF16, tag="z2t", name="z2t")
                nc.vector.tensor_tensor(out=z2t[:], in0=tok2[:],
                                        in1=zbc[:, 0:512], op=ALU.mult)
                z2 = wp.tile([128, 512], BF16, tag="z2", name="z2")
                nc.vector.tensor_tensor(out=z2[:], in0=z2t[:],
                                        in1=zbc[:, 512:1024], op=ALU.add)
                gs = []
                for pair in range(2):
                    fp = p1024.tile([128, 1024], F32, tag="p1024", name="fp")
                    for i in range(2):
                        c4 = 2 * pair + i
                        nc.tensor.matmul(fp[:, 512 * i:512 * (i + 1)],
                                         lhsT=cs[f"wf1T_{c4}"][:], rhs=z2[:],
                                         start=True, stop=True)
                    for i in range(2):
                        c4 = 2 * pair + i
                        gt = wp.tile([128, 512], BF16, tag="gt", name="gt",
                                     bufs=4)
                        nc.scalar.activation(gt[:], fp[:, 512 * i:512 * (i + 1)],
                                             AF.Gelu,
                                             bias=cs[f"bff1_{c4}"][:, 0:1])
                        gs.append(gt)
                f2p = pacc.tile([128, 512], F32, tag="pacc", name="f2p")
                for c4 in range(4):
                    nc.tensor.matmul(f2p[:], lhsT=cs[f"wf2T_{c4}"][:],
                                     rhs=gs[c4][:], start=(c4 == 0),
                                     stop=(c4 == 3))
                tok3 = wp.tile([128, 512], BF16, tag="tok3", name="tok3")
                nc.vector.scalar_tensor_tensor(
                    out=tok3[:], in0=f2p[:], scalar=cs["bff2_col"][:, 0:1],
                    in1=tok2[:], op0=ALU.add, op1=ALU.add)
                msbc = wp.tile([128, 512], BF16, tag="msbc", name="msbc")
                nc.sync.dma_start(msbc[:],
                                  dt["msbig"][:, 512 * g:512 * (g + 1)])
                tok3m = wp.tile([128, 512], BF16, tag="tok3m", name="tok3m")
                nc.vector.tensor_tensor(out=tok3m[:], in0=tok3[:],
                                        in1=msbc[:], op=ALU.mult)
                nc.vector.tensor_reduce(
                    out=xcn_all[:, 16 * g:16 * (g + 1)],
                    in_=tok3m[:].rearrange("p (e k) -> p e k", k=32),
                    axis=mybir.AxisListType.X, op=ALU.add)

            for g0 in range(0, nst, GRP):
                gs_ = list(range(g0, min(g0 + GRP, nst)))
                d1 = {g: s1(g) for g in gs_}
                r1 = {g: s2(g, d1[g][1], "1") for g in gs_}
                d3 = {g: s3(g, d1[g][0], r1[g]) for g in gs_}
                r2 = {g: s2(g, d3[g][1], "2") for g in gs_}
                for g in gs_:
                    s5(g, d3[g][0], r2[g])

            # ---------------- PHASE C: edge MLPs -------------------------
            xcn_bf = cp.tile([128, tcn], BF16, tag="xcn_bf")
            nc.vector.tensor_copy(out=xcn_bf[:], in_=xcn_all[:])

            def dense(rhs_tiles, win, bin_, act, n_ic, out_tag, w):
                outs = []
                for oc in range(2):
                    o = mp.tile([128, w], BF16, tag=f"{out_tag}{oc}",
                                name=out_tag)
                    for nh in range(max(1, w // 512)):
                        cw = min(512, w)
                        p5 = p1024.tile([128, 1024], F32, tag="p1024",
                                        name="p5")
                        for ic in range(n_ic):
                            wt = cs[win(ic, oc)]
                            r = (rhs_tiles if n_ic == 1 else rhs_tiles[ic])
                            nc.tensor.matmul(
                                p5[:, :cw], lhsT=wt[:],
                                rhs=r[:, cw * nh:cw * (nh + 1)],
                                start=(ic == 0), stop=(ic == n_ic - 1))
                        nc.scalar.activation(
                            o[:, cw * nh:cw * (nh + 1)], p5[:, :cw], act,
                            bias=cs[bin_(oc)][:, 0:1])
                    outs.append(o)
                return outs

            def _phase_c(lo, w):
                h1 = dense(xcn_bf[:, lo:lo + w], lambda ic, oc: f"wx1_{oc}",
                           lambda oc: f"bx1_{oc}", AF.Relu, 1, "h1_", w)
                h2 = dense(h1, lambda ic, oc: f"wx2_{ic}{oc}",
                           lambda oc: f"bx2_{oc}", AF.Relu, 2, "h2_", w)
                h3 = dense(h2, lambda ic, oc: f"wx3_{ic}{oc}",
                           lambda oc: f"bx3_{oc}", AF.Identity, 2, "h3_", w)
                j1 = dense(xijT_all[:, lo:lo + w],
                           lambda ic, oc: f"wxj1_{oc}",
                           lambda oc: f"bxj1_{oc}", AF.Relu, 1, "j1_", w)
                j2 = dense(j1, lambda ic, oc: f"wxj2_{ic}{oc}",
                           lambda oc: f"bxj2_{oc}", AF.Identity, 2, "j2_", w)
                zi = []
                for oc in range(2):
                    z = mp.tile([128, w], BF16, tag=f"zi{oc}", name="zi")
                    nc.vector.scalar_tensor_tensor(
                        out=z[:], in0=h3[oc][:], scalar=cs["beta_col"][:, 0:1],
                        in1=j2[oc][:], op0=ALU.mult, op1=ALU.add)
                    zi.append(z)
                zz = dense(zi, lambda ic, oc: f"wl1_{ic}{oc}",
                           lambda oc: f"bl1_{oc}", AF.Relu, 2, "zz", w)
                osb = mp.tile([1, w], BF16, tag="osb", name="osb")
                cw = min(512, w)
                for nh in range(max(1, w // 512)):
                    fo = pacc.tile([128, 512], F32, tag="pacc", name="fo")
                    nc.tensor.matmul(fo[0:1, :cw], lhsT=cs["wl2_0"][:],
                                     rhs=zz[0][:, cw * nh:cw * (nh + 1)],
                                     start=True, stop=False)
                    nc.tensor.matmul(fo[0:1, :cw], lhsT=cs["wl2_1"][:],
                                     rhs=zz[1][:, cw * nh:cw * (nh + 1)],
                                     start=False, stop=True)
                    nc.scalar.activation(osb[0:1, cw * nh:cw * (nh + 1)],
                                         fo[0:1, :cw], AF.Identity,
                                         bias=cs["bl2"][0:1, 0:1])
                osf = mp.tile([1, w], F32, tag="osf", name="osf")
                nc.vector.tensor_copy(out=osf[:], in_=osb[:])
                nc.sync.dma_start(out_dram[0:1, lo:lo + w], osf[:])

            for lo in range(0, tcn, 512):
                _phase_c(lo, min(512, tcn - lo))

    nc.finalize()
    return nc


# ---------------------------------------------------------------- host side

def _prep_shared(inp):
    f = lambda k: np.asarray(inp[k], np.float32)
    tok_w, tok_b = f("tok_w"), f("tok_b")
    g1, b1 = f("ln1_g"), f("ln1_b")
    qkv_w, qkv_b = f("qkv_w"), f("qkv_b")
    out_w, out_b = f("out_w"), f("out_b")
    g2, b2 = f("ln2_g"), f("ln2_b")
    ff1_w, ff1_b = f("ff1_w"), f("ff1_b")
    ff2_w, ff2_b = f("ff2_w"), f("ff2_b")

    d = {}
    d["w0xT"] = bfa(tok_w[:, :C].T)
    d["a1"] = bfa(tok_w[:, C:2 * C].T)
    d["a2"] = bfa(tok_w[:, 2 * C:3 * C].T)
    d["a3"] = bfa(tok_w[:, 3 * C:4 * C].T)
    d["tokb_row"] = bfa(tok_b[None, :])

    sc = 1.0 / np.sqrt(DH)
    Wq, Wk, Wv = qkv_w[:C], qkv_w[C:2 * C], qkv_w[2 * C:3 * C]
    bq, bk, bv = qkv_b[:C], qkv_b[C:2 * C], qkv_b[2 * C:3 * C]
    Wq_e = Wq * g1[None, :] * sc
    bq_e = (Wq @ b1) * sc + bq * sc
    Wk_e = Wk * g1[None, :]
    Wv_e = Wv * g1[None, :]
    bv_e = Wv @ b1 + bv
    w_ck = Wk_e.T @ bq_e
    d["wqT"] = bfa(Wq_e.T)
    d["wkT"] = bfa(Wk_e.T)
    d["wv_aug"] = bfa(np.concatenate([Wv_e.T, w_ck[:, None]], axis=1))
    ones = np.ones(C, np.float32)
    d["wq1_rep"] = bfa(np.tile((Wq_e @ ones)[None, :], (128, 1)))
    d["wk1_rep"] = bfa(np.tile((Wk_e @ ones)[None, :], (128, 1)))
    d["wv1_rep"] = bfa(np.tile(np.concatenate(
        [Wv_e @ ones, [w_ck @ ones]])[None, :], (128, 1)))

    for nm, heads in (("woutA", [0, 1, 2, 3]), ("woutB", [4, 5, 6, 7])):
        w = np.zeros((128, 128), np.float32)
        for i, h in enumerate(heads):
            w[32 * i:32 * i + 16, :] = out_w[:, 16 * h:16 * h + 16].T
        d[nm] = bfa(w)
    d["outb_col"] = f32a((out_b + out_w @ bv_e)[:, None])

    for c4 in range(4):
        sl = slice(128 * c4, 128 * (c4 + 1))
        d[f"wf1T_{c4}"] = bfa((ff1_w[sl, :] * g2[None, :]).T)
        d[f"bff1_{c4}"] = f32a((ff1_w[sl, :] @ b2 + ff1_b[sl])[:, None])
        d[f"wf2T_{c4}"] = bfa(ff2_w[:, sl].T)
    d["bff2_col"] = f32a(ff2_b[:, None])

    for nm, wkey, bkey in (("wx1", "xcn_w1", "xcn_b1"),
                           ("wxj1", "xij_w1", "xij_b1")):
        W, B = f(wkey), f(bkey)
        for oc in range(2):
            sl = slice(128 * oc, 128 * (oc + 1))
            d[f"{nm}_{oc}"] = bfa(W[sl, :].T)
            d[f"b{nm[1:]}_{oc}"] = f32a(B[sl][:, None])
    for nm, wkey, bkey in (("wx2", "xcn_w2", "xcn_b2"),
                           ("wx3", "xcn_w3", "xcn_b3"),
                           ("wxj2", "xij_w2", "xij_b2"),
                           ("wl1", "lin_w1", "lin_b1")):
        W, B = f(wkey), f(bkey)
        for ic in range(2):
            for oc in range(2):
                d[f"{nm}_{ic}{oc}"] = bfa(
                    W[128 * oc:128 * (oc + 1), 128 * ic:128 * (ic + 1)].T)
        for oc in range(2):
            d[f"b{nm[1:]}_{oc}"] = f32a(B[128 * oc:128 * (oc + 1)][:, None])
    lin_w2, lin_b2 = f("lin_w2"), f("lin_b2")
    d["wl2_0"] = bfa(lin_w2[0, :128][:, None])
    d["wl2_1"] = bfa(lin_w2[0, 128:][:, None])
    d["bl2"] = f32a(lin_b2.reshape(1, 1))

    Bind = np.zeros((16, 512), np.float32)
    for e in range(16):
        Bind[e, 32 * e:32 * (e + 1)] = 1.0
    d["Bind"] = bfa(Bind)
    bm = np.zeros((128, 1024), np.float32)
    for h in range(8):
        for e in range(4):
            bm[32 * e:32 * (e + 1), 128 * h + 32 * e:128 * h + 32 * (e + 1)] = 1.0
    d["bandmask"] = bfa(bm)
    d["wmean"] = bfa(np.full((128, 1), 1.0 / 128.0))
    d["ones_rep"] = bfa(np.ones((128, 128)))
    d["eps_col"] = f32a(np.full((128, 1), 1e-5))
    d["epsd_col"] = f32a(np.full((128, 1), 1e-30))
    d["beta_col"] = f32a(np.full((128, 1),
                                 np.asarray(inp["beta"],
                                            np.float32).reshape(-1)[0]))
    return d


def _prep_core(inp, core, nst=NST):
    sl = slice(core * TC, (core + 1) * TC)
    tar = np.asarray(inp["tar_ei"])[:, sl].astype(np.int32)
    cols = np.asarray(inp["cn_cols"])[sl].astype(np.int32)     # [TC, K]
    cnt = np.asarray(inp["cn_counts"])[sl].astype(np.int64)    # [TC]

    d = {}
    nsub = TC * K // 128   # 256 subtiles
    d["idx_cn"] = np.ascontiguousarray(
        cols.reshape(-1).reshape(nsub, 128).T)[:, :4 * NST].copy()
    ne = TC // 128
    d["idx_t0"] = np.ascontiguousarray(tar[0].reshape(ne, 128).T)
    d["idx_t1"] = np.ascontiguousarray(tar[1].reshape(ne, 128).T)

    kk = np.arange(K)
    valid = (kk[None, :] < cnt[:, None]).reshape(-1)           # [TC*K]
    vcol = np.ascontiguousarray(valid.reshape(nsub, 128).T)
    d["negv"] = ((~vcol) * np.float32(NEG)).astype(np.float32)[:, :4 * NST].copy()

    ms = valid.astype(np.float32) / np.maximum(cnt, 1).astype(
        np.float32).repeat(K)
    d["msbig"] = np.ascontiguousarray(
        np.broadcast_to(bfa(ms[None, :512 * NST]), (128, 512 * NST)))
    return d


_CACHE = {}
_CACHE_LOCK = threading.Lock()


def _get_nc(nst=NST):
    with _CACHE_LOCK:
        if nst not in _CACHE:
            _CACHE[nst] = _build_nc(nst)
        return _CACHE[nst]


def run(inputs, nst=NST, **spmd_kwargs):
    nc = _get_nc(nst)
    shared = _prep_shared(inputs)
    xbf = np.ascontiguousarray(
        np.asarray(inputs["x"], np.float32)).astype(ml_dtypes.bfloat16)
    in_maps = []
    for core in range(NCORES):
        m = dict(shared)
        m["xbf"] = xbf
        m.update(_prep_core(inputs, core, nst))
        in_maps.append(m)
    res = run_bass_kernel_spmd(nc, in_maps, core_ids=list(range(NCORES)),
                               **spmd_kwargs)
    out = np.stack([res.results[c]["out"][0] for c in range(NCORES)])
    return out, res


def kernel(**inputs):
    out, _ = run(inputs)
    return out.reshape(T, O).astype(np.float32)


# revision 19
# speedup vs baseline: 2.2855x; 1.0148x over previous
"""Trainium2 Bass kernel for nn_CNLinkPredictor (gnn_message_passing), v2.

Data-parallel over target edges T (8192) across 8 NeuronCores (1024
edges/core).  v2 reworks the baseline around three findings from the HW
profile: fp32 matmuls run as 2 half-rate passes (4 cyc/row vs 1 for bf16),
ACT/DVE per-instruction overheads demand 512-wide ops, and ACT table
switches cost 1.3us.

Layout: channel-major [128 ch, 512 tok] supertiles (16 edges each);
 - all matmuls bf16 (PSUM f32); x is pre-cast to bf16 in DRAM.
 - LN has no affine (folded into following weights); stats are computed
   with per-token-column outputs via tok-chunk-stationary matmuls, the
   scalar math runs on [128,4] tiles, and rstd/-mu*rstd rows are
   transposed once and broadcast with rank-1 ones matmuls.
 - mean-subtraction for the q/k/v projections is folded into rank-1
   PSUM corrections (W@1 outer -mu*rstd), so z1 is just tok*rstd_bc.
 - per-key softmax bias exp(c_k) and key-validity are folded into a
   post-exp scaling eps of V (and the denominator aug channel); the
   cross-edge block mask is a constant bf16 0/1 multiply on E.
 - scores use a band-structured q ("qbds", built with SBUF->SBUF DMAs,
   zeros persistent) against dense channel-major k slices.
 - pooling = masked (valid/cnt) multiply + segmented DVE reduce.
 - phases grouped G=8 supertiles so ACT tables (sqrt/exp/gelu) load
   once per group.
"""

import sys
import threading

sys.path.insert(0, "/opt/trn_rl_repo")

import numpy as np
import ml_dtypes

import concourse.bass as bass
import concourse.bacc as bacc
import concourse.mybir as mybir
from concourse.tile import TileContext
from concourse.masks import make_identity
from concourse.bass_utils import run_bass_kernel_spmd

F32 = mybir.dt.float32
BF16 = mybir.dt.bfloat16
I32 = mybir.dt.int32
AF = mybir.ActivationFunctionType
ALU = mybir.AluOpType

N, C, H, O = 100000, 128, 256, 1
T, K = 8192, 32
NHEAD, DH, FF = 8, 16, 512
NCORES = 8
TC = T // NCORES          # 1024 edges per core
NST = TC // 16            # 64 supertiles (512 tokens / 16 edges each)
GRP = 8                   # supertiles per table-phase group
NEG = -1e9

bfa = lambda a: np.ascontiguousarray(np.asarray(a, np.float32)).astype(ml_dtypes.bfloat16)
f32a = lambda a: np.ascontiguousarray(np.asarray(a, np.float32))


def _build_nc(nst=NST):
    nc = bacc.Bacc("TRN2", target_bir_lowering=False)
    tcn = 16 * nst                     # edges this build covers per core
    ne = TC // 128                     # 8 phase-A tiles (always full)

    dt = {}

    def din(name, shape, dtype=BF16):
        dt[name] = nc.dram_tensor(name, shape, dtype, kind="ExternalInput")
        return dt[name]

    # data
    din("xbf", [N, C])
    din("idx_cn", [128, 4 * NST], I32)
    din("idx_t0", [128, ne], I32)
    din("idx_t1", [128, ne], I32)
    din("negv", [128, 4 * NST], F32)
    din("msbig", [128, 512 * NST])
    # weights
    for nm in ["w0xT", "a1", "a2", "a3", "wqT", "wkT", "woutA", "woutB",
               "wf1T_0", "wf1T_1", "wf1T_2", "wf1T_3",
               "wf2T_0", "wf2T_1", "wf2T_2", "wf2T_3",
               "wx1_0", "wx1_1", "wxj1_0", "wxj1_1"]:
        din(nm, [128, 128])
    for ic in range(2):
        for oc in range(2):
            for nm in ["wx2", "wx3", "wxj2", "wl1"]:
                din(f"{nm}_{ic}{oc}", [128, 128])
    din("wv_aug", [128, 129])
    din("wl2_0", [128, 1])
    din("wl2_1", [128, 1])
    din("Bind", [16, 512])
    din("bandmask", [128, 1024])
    din("wmean", [128, 1])
    din("ones_rep", [128, 128])
    din("tokb_row", [1, 128])
    for nm in ["outb_col", "bff2_col", "eps_col", "epsd_col", "beta_col",
               "bx1_0", "bx1_1", "bx2_0", "bx2_1", "bx3_0", "bx3_1",
               "bxj1_0", "bxj1_1", "bxj2_0", "bxj2_1", "bl1_0", "bl1_1"]:
        din(nm, [128, 1], F32)
    for c4 in range(4):
        din(f"bff1_{c4}", [128, 1], F32)
    din("bl2", [1, 1], F32)

    ec_dram = nc.dram_tensor("ec_dram", [TC, 128], BF16)
    zrow_dram = nc.dram_tensor("zrow_dram", [NST, 2048], BF16)
    out_dram = nc.dram_tensor("out", [1, tcn], F32, kind="ExternalOutput")

    with TileContext(nc) as tc:
        with (
            nc.allow_low_precision(reason="bf16 pipeline validated vs ref"),
            tc.tile_pool(name="cpool", bufs=1) as cp,
            tc.tile_pool(name="wp", bufs=2) as wp,
            tc.tile_pool(name="mp", bufs=2) as mp,
            tc.tile_pool(name="p1024", bufs=2, space="PSUM") as p1024,
            tc.tile_pool(name="pacc", bufs=1, space="PSUM") as pacc,
            tc.tile_pool(name="pT", bufs=1, space="PSUM") as pTp,
            tc.tile_pool(name="pctx", bufs=2, space="PSUM") as pctxp,
        ):
            cs = {}
            for nm, t in dt.items():
                if nm in ("xbf", "msbig"):
                    continue
                tile = cp.tile(list(t.shape), t.dtype, tag=f"c_{nm}", name=nm)
                nc.sync.dma_start(tile[:], t[:])
                cs[nm] = tile

            ident = cp.tile([128, 128], BF16, tag="ident")
            make_identity(nc, ident[:])
            identf = cp.tile([128, 128], F32, tag="identf")
            make_identity(nc, identf[:])

            xijT_all = cp.tile([128, TC], BF16, tag="xijT_all")
            xcn_all = cp.tile([128, tcn], F32, tag="xcn_all")
            qbds = cp.tile([128, 4096], BF16, tag="qbds")
            nc.vector.memset(qbds[:], 0.0)

            # ---------------- PHASE A: per-edge EC + xijT ----------------
            for j in range(ne):
                xi = wp.tile([128, C], BF16, tag="xi")
                xj = wp.tile([128, C], BF16, tag="xj")
                nc.gpsimd.indirect_dma_start(
                    out=xi[:], out_offset=None, in_=dt["xbf"][:],
                    in_offset=bass.IndirectOffsetOnAxis(
                        ap=cs["idx_t0"][:, j:j + 1], axis=0))
                nc.gpsimd.indirect_dma_start(
                    out=xj[:], out_offset=None, in_=dt["xbf"][:],
                    in_offset=bass.IndirectOffsetOnAxis(
                        ap=cs["idx_t1"][:, j:j + 1], axis=0))
                xij = wp.tile([128, C], BF16, tag="xij")
                nc.vector.tensor_tensor(out=xij[:], in0=xi[:], in1=xj[:],
                                        op=ALU.mult)
                pt = pTp.tile([128, 512], BF16, tag="pT", name="pt")
                nc.tensor.transpose(pt[:, 0:128], xi[:], ident[:])
                nc.tensor.transpose(pt[:, 128:256], xj[:], ident[:])
                nc.tensor.transpose(pt[:, 256:384], xij[:], ident[:])
                xiT = wp.tile([128, 128], BF16, tag="xiT")
                nc.vector.tensor_copy(out=xiT[:], in_=pt[:, 0:128])
                xjT = wp.tile([128, 128], BF16, tag="xjT")
                nc.vector.tensor_copy(out=xjT[:], in_=pt[:, 128:256])
                nc.vector.tensor_copy(out=xijT_all[:, 128 * j:128 * (j + 1)],
                                      in_=pt[:, 256:384])

                ecp = pctxp.tile([128, 258], F32, tag="pctx", name="ecp")
                nc.tensor.matmul(ecp[:, 0:128], lhsT=xiT[:], rhs=cs["a1"][:],
                                 start=True, stop=False)
                nc.tensor.matmul(ecp[:, 0:128], lhsT=xjT[:], rhs=cs["a2"][:],
                                 start=False, stop=False)
                nc.tensor.matmul(ecp[:, 0:128],
                                 lhsT=xijT_all[:, 128 * j:128 * (j + 1)],
                                 rhs=cs["a3"][:], start=False, stop=False)
                nc.tensor.matmul(ecp[:, 0:128], lhsT=cs["ones_rep"][0:1, :],
                                 rhs=cs["tokb_row"][:], start=False, stop=True)
                ec_s = wp.tile([128, 128], BF16, tag="ec_s")
                nc.vector.tensor_copy(out=ec_s[:], in_=ecp[:, 0:128])
                nc.sync.dma_start(ec_dram[128 * j:128 * (j + 1), :], ec_s[:])

            # ---------------- PHASE B: grouped supertiles ----------------
            SHUF16 = [16] * 32

            def _stats(t_a, t_b):
                """mean rows via 2 matmuls -> transpose -> [128,{mu4|ms2 4}]."""
                strows = p1024.tile([128, 1024], F32, tag="p1024",
                                    name="strows")
                nc.tensor.matmul(strows[0:1, 0:512], lhsT=cs["wmean"][:],
                                 rhs=t_a[:], start=True, stop=True)
                nc.tensor.matmul(strows[0:1, 512:1024], lhsT=cs["wmean"][:],
                                 rhs=t_b[:], start=True, stop=True)
                rows33 = wp.tile([33, 512], F32, tag="rows33", name="rows33")
                nc.vector.tensor_copy(out=rows33[0:1, :],
                                      in_=strows[0:1, 0:512])
                nc.vector.tensor_copy(out=rows33[32:33, :],
                                      in_=strows[0:1, 512:1024])
                stx = pctxp.tile([128, 258], F32, tag="pctx", name="stx")
                for c in range(2):
                    nc.tensor.transpose(stx[:, 66 * c:66 * c + 33],
                                        rows33[:, 256 * c:256 * c + 128],
                                        identf[0:33, 0:33])
                    nc.tensor.transpose(stx[:, 66 * c + 33:66 * c + 66],
                                        rows33[:, 256 * c + 128:256 * (c + 1)],
                                        identf[0:33, 0:33])
                st_sb = wp.tile([128, 8], F32, tag="st_sb", name="st_sb",
                                bufs=9)
                sv = stx[:, 0:132].rearrange("p (c o) -> p c o", o=33)
                nc.vector.tensor_copy(
                    out=st_sb[:, 0:4].rearrange("p (c o) -> p c o", o=1),
                    in_=sv[:, :, 0:1])
                nc.vector.tensor_copy(
                    out=st_sb[:, 4:8].rearrange("p (c o) -> p c o", o=1),
                    in_=sv[:, :, 32:33])
                return st_sb

            def s1(g):
                """gather + transpose + tok(relu) + LN1 raw stats."""
                xw = wp.tile([128, 512], BF16, tag="xw", name="xw")
                for s in range(4):
                    nc.gpsimd.indirect_dma_start(
                        out=xw[:, 128 * s:128 * (s + 1)], out_offset=None,
                        in_=dt["xbf"][:],
                        in_offset=bass.IndirectOffsetOnAxis(
                            ap=cs["idx_cn"][:, 4 * g + s:4 * g + s + 1],
                            axis=0))
                pt = pTp.tile([128, 512], BF16, tag="pT", name="pt")
                for s in range(4):
                    nc.tensor.transpose(pt[:, 128 * s:128 * (s + 1)],
                                        xw[:, 128 * s:128 * (s + 1)], ident[:])
                xwcm = wp.tile([128, 512], BF16, tag="xwcm", name="xwcm")
                nc.vector.tensor_copy(out=xwcm[:], in_=pt[:])
                ec16 = wp.tile([16, 128], BF16, tag="ec16", name="ec16")
                nc.sync.dma_start(ec16[:], ec_dram[16 * g:16 * (g + 1), :])
                tokp = pacc.tile([128, 512], F32, tag="pacc", name="tokp")
                nc.tensor.matmul(tokp[:], lhsT=cs["w0xT"][:], rhs=xwcm[:],
                                 start=True, stop=False)
                nc.tensor.matmul(tokp[:], lhsT=ec16[:], rhs=cs["Bind"][:],
                                 start=False, stop=True)
                tok = wp.tile([128, 512], BF16, tag="tok", name="tok", bufs=9)
                nc.scalar.activation(tok[:], tokp[:], AF.Relu)
                sq = wp.tile([128, 512], BF16, tag="sq", name="sq")
                nc.vector.tensor_tensor(out=sq[:], in0=tok[:], in1=tok[:],
                                        op=ALU.mult)
                st_sb = _stats(tok, sq)
                return tok, st_sb

            def s2(g, st_sb, tag):
                """[128,4] scalar math -> rowT [8,128] = {rstd | -mu*rstd}."""
                mu = st_sb[:, 0:4]
                rows = wp.tile([128, 8], F32, tag="rows" + tag, name="rows")
                musq = wp.tile([128, 4], F32, tag="musq" + tag, name="musq")
                nc.vector.tensor_tensor(out=musq[:], in0=mu, in1=mu,
                                        op=ALU.mult)
                varr = wp.tile([128, 4], F32, tag="varr" + tag, name="varr")
                nc.vector.tensor_tensor(out=varr[:], in0=st_sb[:, 4:8],
                                        in1=musq[:], op=ALU.subtract)
                stdd = wp.tile([128, 4], F32, tag="stdd" + tag, name="stdd")
                nc.scalar.activation(stdd[:], varr[:], AF.Sqrt,
                                     bias=cs["eps_col"][:, 0:1])
                nc.vector.reciprocal(rows[:, 0:4], stdd[:])
                negmu = wp.tile([128, 4], F32, tag="negmu" + tag, name="negmu")
                nc.vector.tensor_scalar(out=negmu[:], in0=mu, scalar1=-1.0,
                                        scalar2=None, op0=ALU.mult)
                nc.vector.tensor_tensor(out=rows[:, 4:8], in0=negmu[:],
                                        in1=rows[:, 0:4], op=ALU.mult)
                # place the 8 per-subtile scalars into columns whose
                # transposed rows land on legal PE base partitions (0/32/64):
                # s<3: rowT[32s, 0:128]=rstd_s, rowT[32s, 128:256]=-mu*rstd_s
                # s=3: rowT[0, 256:384]=rstd_3, rowT[0, 384:512]=-mu*rstd_3
                rsp = wp.tile([128, 512], BF16, tag="rsp" + tag, name="rsp")
                nc.vector.tensor_copy(
                    out=rsp[:, 0:96].rearrange("p (s o) -> p s o", o=32)[:, :, 0:1],
                    in_=rows[:, 0:3].rearrange("p (s o) -> p s o", o=1))
                nc.vector.tensor_copy(
                    out=rsp[:, 128:224].rearrange("p (s o) -> p s o", o=32)[:, :, 0:1],
                    in_=rows[:, 4:7].rearrange("p (s o) -> p s o", o=1))
                nc.vector.tensor_copy(out=rsp[:, 256:257], in_=rows[:, 3:4])
                nc.vector.tensor_copy(out=rsp[:, 384:385], in_=rows[:, 7:8])
                pt = pTp.tile([128, 512], BF16, tag="pT", name="pt")
                for c in range(4):
                    nc.tensor.transpose(pt[:, 128 * c:128 * (c + 1)],
                                        rsp[:, 128 * c:128 * (c + 1)],
                                        ident[:])
                rsb = wp.tile([65, 512], BF16, tag="rsb" + tag, name="rsb")
                nc.vector.tensor_copy(out=rsb[:], in_=pt[0:65, :])
                # stage through DRAM: zrow[g] = [rstd 512 | -mu*rstd 512]
                zoff = 0 if tag == "1" else 1024
                zr = zrow_dram[g:g + 1, zoff:zoff + 1024]
                for s in range(4):
                    row, c0 = (32 * s, 0) if s < 3 else (0, 256)
                    sap = rsb[row:row + 1, c0:c0 + 256]
                    srcw = bass.AP(tensor=sap.tensor, offset=sap.offset,
                                   ap=[sap.ap[0], [128, 2], [1, 128]])
                    dstw = bass.AP(tensor=zr.tensor,
                                   offset=zr.offset + 128 * s,
                                   ap=[[2048, 1], [512, 2], [1, 128]])
                    eng = nc.sync if s % 2 == 0 else nc.gpsimd
                    eng.dma_start(dstw, srcw)
                zbc_sb = wp.tile([128, 1024], BF16, tag="zbc" + tag,
                                 name="zbc_sb", bufs=9)
                for j in range(2):
                    sap = zrow_dram[g:g + 1, zoff + 512 * j:zoff + 512 * (j + 1)]
                    srcb = bass.AP(tensor=sap.tensor, offset=sap.offset,
                                   ap=[[0, 128], [1, 512]])
                    eng = nc.sync if j == 0 else nc.gpsimd
                    eng.dma_start(zbc_sb[:, 512 * j:512 * (j + 1)], srcb)
                return zbc_sb

            def s3(g, tok, zbc_sb):
                """attention + out-proj + residual + LN2 raw stats."""
                z1t = wp.tile([128, 512], BF16, tag="z1t", name="z1t")
                nc.vector.tensor_tensor(out=z1t[:], in0=tok[:],
                                        in1=zbc_sb[:, 0:512], op=ALU.mult)
                z1 = wp.tile([128, 512], BF16, tag="z1", name="z1")
                nc.vector.tensor_tensor(out=z1[:], in0=z1t[:],
                                        in1=zbc_sb[:, 512:1024], op=ALU.add)
                qkp = p1024.tile([128, 1024], F32, tag="p1024", name="qkp")
                nc.tensor.matmul(qkp[:, 0:512], lhsT=cs["wqT"][:], rhs=z1[:],
                                 start=True, stop=True)
                nc.tensor.matmul(qkp[:, 512:1024], lhsT=cs["wkT"][:],
                                 rhs=z1[:], start=True, stop=True)
                qk = wp.tile([128, 1024], BF16, tag="qk", name="qk")
                nc.scalar.copy(qk[:, 0:512], qkp[:, 0:512])
                nc.scalar.copy(qk[:, 512:1024], qkp[:, 512:1024])
                # qbds bands via sbuf->sbuf DMA (zeros persistent)
                for h in range(8):
                    nc.sync.dma_start(
                        qbds[16 * h:16 * h + 16, :].rearrange(
                            "p (s hh q) -> p s hh q", hh=8, q=128)[:, :, h, :],
                        qk[16 * h:16 * h + 16, 0:512].rearrange(
                            "p (s q) -> p s q", q=128))
                # v + eps + vaug per subtile
                vaugs = []
                for p2 in range(2):
                    vp = pctxp.tile([128, 258], F32, tag="pctx", name="vp")
                    for i in range(2):
                        s = 2 * p2 + i
                        nc.tensor.matmul(vp[:, 129 * i:129 * i + 129],
                                         lhsT=z1[:, 128 * s:128 * (s + 1)],
                                         rhs=cs["wv_aug"][:],
                                         start=True, stop=True)
                    for i in range(2):
                        s = 2 * p2 + i
                        v_sb = wp.tile([128, 129], BF16, tag="v_sb",
                                       name="v_sb", bufs=4)
                        nc.vector.tensor_copy(out=v_sb[:],
                                              in_=vp[:, 129 * i:129 * i + 129])
                        epsc = wp.tile([128, 1], F32, tag="epsc", name="epsc",
                                       bufs=4)
                        nc.scalar.activation(
                            epsc[:], v_sb[:, 128:129], AF.Exp,
                            bias=cs["negv"][:, 4 * g + s:4 * g + s + 1])
                        vaug = wp.tile([128, 136], BF16, tag="vaug",
                                       name="vaug", bufs=4)
                        vv = vaug[:].rearrange("p (h d) -> p h d", d=17)
                        nc.vector.tensor_scalar(
                            out=vv[:, :, 0:16],
                            in0=v_sb[:, 0:128].rearrange("p (h d) -> p h d",
                                                         d=16),
                            scalar1=epsc[:, 0:1], scalar2=None, op0=ALU.mult)
                        nc.gpsimd.tensor_copy(
                            out=vv[:, :, 16:17],
                            in_=epsc[:, 0:1].to_broadcast([128, 8, 1]))
                        vaugs.append(vaug)
                # scores -> exp -> mask -> ctx -> norm, per subtile
                ups = pacc.tile([128, 512], F32, tag="pacc", name="ups")
                cn_all = wp.tile([128, 1024], BF16, tag="cn_all",
                                 name="cn_all")
                for s in range(4):
                    scp = p1024.tile([128, 1024], F32, tag="p1024", name="scp")
                    for half in range(2):
                        nc.tensor.matmul(
                            scp[:, 512 * half:512 * (half + 1)],
                            lhsT=qk[:, 512 + 128 * s:512 + 128 * (s + 1)],
                            rhs=qbds[:, 1024 * s + 512 * half:
                                     1024 * s + 512 * (half + 1)],
                            start=True, stop=True)
                    E = wp.tile([128, 1024], BF16, tag="E", name="E")
                    nc.scalar.activation(E[:, 0:512], scp[:, 0:512], AF.Exp)
                    nc.scalar.activation(E[:, 512:1024], scp[:, 512:1024],
                                         AF.Exp)
                    Em = wp.tile([128, 1024], BF16, tag="Em", name="Em")
                    nc.vector.tensor_tensor(out=Em[:], in0=E[:],
                                            in1=cs["bandmask"][:],
                                            op=ALU.mult)
                    ctxp = pctxp.tile([128, 258], F32, tag="pctx", name="ctxp")
                    for h in range(8):
                        co = 128 * (h // 4)
                        hh = 32 * (h % 4)
                        nc.tensor.matmul(
                            ctxp[hh:hh + 17, co:co + 128],
                            lhsT=vaugs[s][:, 17 * h:17 * h + 17],
                            rhs=Em[:, 128 * h:128 * (h + 1)],
                            start=True, stop=True, tile_position=(0, hh))
                    cx = wp.tile([128, 256], F32, tag="cx", name="cx")
                    nc.vector.tensor_scalar(out=cx[:], in0=ctxp[:, 0:256],
                                            scalar1=cs["epsd_col"][:, 0:1],
                                            scalar2=None, op0=ALU.add)
                    rt = wp.tile([128, 256], F32, tag="rt", name="rt")
                    nc.vector.stream_shuffle(rt[:], cx[:], SHUF16)
                    rtr = wp.tile([128, 256], F32, tag="rtr", name="rtr")
                    nc.vector.reciprocal_approx_fast(rtr[:], rt[:])
                    nc.vector.tensor_tensor(
                        out=cn_all[:, 256 * s:256 * (s + 1)], in0=cx[:],
                        in1=rtr[:], op=ALU.mult)
                cv = cn_all[:].rearrange("p (s2 hf q) -> p s2 hf q",
                                         s2=4, hf=2)
                nc.tensor.matmul(ups[:], lhsT=cs["woutA"][:],
                                 rhs=cv[:, :, 0, :], start=True, stop=False)
                nc.tensor.matmul(ups[:], lhsT=cs["woutB"][:],
                                 rhs=cv[:, :, 1, :], start=False, stop=True)
                tok2 = wp.tile([128, 512], BF16, tag="tok2", name="tok2",
                               bufs=9)
                nc.vector.scalar_tensor_tensor(
                    out=tok2[:], in0=ups[:], scalar=cs["outb_col"][:, 0:1],
                    in1=tok[:], op0=ALU.add, op1=ALU.add)
                sq2 = wp.tile([128, 512], BF16, tag="sq2", name="sq2")
                nc.vector.tensor_tensor(out=sq2[:], in0=tok2[:], in1=tok2[:],
                                        op=ALU.mult)
                st2_sb = _stats(tok2, sq2)
                return tok2, st2_sb

            def s5(g, tok2, zbc2_sb):
                """LN2 apply + FF + residual + masked pool -> xcn_all."""
                z2t = wp.tile([128, 512], BF16, tag="z2t", name="z2t")
                nc.vector.tensor_tensor(out=z2t[:], in0=tok2[:],
                                        in1=zbc2_sb[:, 0:512], op=ALU.mult)
                z2 = wp.tile([128, 512], BF16, tag="z2", name="z2")
                nc.vector.tensor_tensor(out=z2[:], in0=z2t[:],
                                        in1=zbc2_sb[:, 512:1024], op=ALU.add)
                gs = []
                for pair in range(2):
                    fp = p1024.tile([128, 1024], F32, tag="p1024", name="fp")
                    for i in range(2):
                        c4 = 2 * pair + i
                        nc.tensor.matmul(fp[:, 512 * i:512 * (i + 1)],
                                         lhsT=cs[f"wf1T_{c4}"][:], rhs=z2[:],
                                         start=True, stop=True)
                    for i in range(2):
                        c4 = 2 * pair + i
                        gt = wp.tile([128, 512], BF16, tag="gt", name="gt",
                                     bufs=4)
                        nc.scalar.activation(gt[:], fp[:, 512 * i:512 * (i + 1)],
                                             AF.Gelu,
                                             bias=cs[f"bff1_{c4}"][:, 0:1])
                        gs.append(gt)
                f2p = pacc.tile([128, 512], F32, tag="pacc", name="f2p")
                for c4 in range(4):
                    nc.tensor.matmul(f2p[:], lhsT=cs[f"wf2T_{c4}"][:],
                                     rhs=gs[c4][:], start=(c4 == 0),
                                     stop=(c4 == 3))
                tok3 = wp.tile([128, 512], BF16, tag="tok3", name="tok3")
                nc.vector.scalar_tensor_tensor(
                    out=tok3[:], in0=f2p[:], scalar=cs["bff2_col"][:, 0:1],
                    in1=tok2[:], op0=ALU.add, op1=ALU.add)
                msbc = wp.tile([128, 512], BF16, tag="msbc", name="msbc")
                nc.sync.dma_start(msbc[:],
                                  dt["msbig"][:, 512 * g:512 * (g + 1)])
                tok3m = wp.tile([128, 512], BF16, tag="tok3m", name="tok3m")
                nc.vector.tensor_tensor(out=tok3m[:], in0=tok3[:],
                                        in1=msbc[:], op=ALU.mult)
                nc.vector.tensor_reduce(
                    out=xcn_all[:, 16 * g:16 * (g + 1)],
                    in_=tok3m[:].rearrange("p (e k) -> p e k", k=32),
                    axis=mybir.AxisListType.X, op=ALU.add)

            for g0 in range(0, nst, GRP):
                gs_ = list(range(g0, min(g0 + GRP, nst)))
                d1 = {g: s1(g) for g in gs_}
                r1 = {g: s2(g, d1[g][1], "1") for g in gs_}
                d3 = {g: s3(g, d1[g][0], r1[g]) for g in gs_}
                r2 = {g: s2(g, d3[g][1], "2") for g in gs_}
                for g in gs_:
                    s5(g, d3[g][0], r2[g])

            # ---------------- PHASE C: edge MLPs -------------------------
            xcn_bf = cp.tile([128, tcn], BF16, tag="xcn_bf")
            nc.vector.tensor_copy(out=xcn_bf[:], in_=xcn_all[:])

            def dense(rhs_tiles, win, bin_, act, n_ic, out_tag, w):
                outs = []
                for oc in range(2):
                    o = mp.tile([128, w], BF16, tag=f"{out_tag}{oc}",
                                name=out_tag)
                    for nh in range(max(1, w // 512)):
                        cw = min(512, w)
                        p5 = p1024.tile([128, 1024], F32, tag="p1024",
                                        name="p5")
                        for ic in range(n_ic):
                            wt = cs[win(ic, oc)]
                            r = (rhs_tiles if n_ic == 1 else rhs_tiles[ic])
                            nc.tensor.matmul(
                                p5[:, :cw], lhsT=wt[:],
                                rhs=r[:, cw * nh:cw * (nh + 1)],
                                start=(ic == 0), stop=(ic == n_ic - 1))
                        nc.scalar.activation(
                            o[:, cw * nh:cw * (nh + 1)], p5[:, :cw], act,
                            bias=cs[bin_(oc)][:, 0:1])
                    outs.append(o)
                return outs

            def _phase_c(lo, w):
                h1 = dense(xcn_bf[:, lo:lo + w], lambda ic, oc: f"wx1_{oc}",
                           lambda oc: f"bx1_{oc}", AF.Relu, 1, "h1_", w)
                h2 = dense(h1, lambda ic, oc: f"wx2_{ic}{oc}",
                           lambda oc: f"bx2_{oc}", AF.Relu, 2, "h2_", w)
                h3 = dense(h2, lambda ic, oc: f"wx3_{ic}{oc}",
                           lambda oc: f"bx3_{oc}", AF.Identity, 2, "h3_", w)
                j1 = dense(xijT_all[:, lo:lo + w],
                           lambda ic, oc: f"wxj1_{oc}",
                           lambda oc: f"bxj1_{oc}", AF.Relu, 1, "j1_", w)
                j2 = dense(j1, lambda ic, oc: f"wxj2_{ic}{oc}",
                           lambda oc: f"bxj2_{oc}", AF.Identity, 2, "j2_", w)
                zi = []
                for oc in range(2):
                    z = mp.tile([128, w], BF16, tag=f"zi{oc}", name="zi")
                    nc.vector.scalar_tensor_tensor(
                        out=z[:], in0=h3[oc][:], scalar=cs["beta_col"][:, 0:1],
                        in1=j2[oc][:], op0=ALU.mult, op1=ALU.add)
                    zi.append(z)
                zz = dense(zi, lambda ic, oc: f"wl1_{ic}{oc}",
                           lambda oc: f"bl1_{oc}", AF.Relu, 2, "zz", w)
                osb = mp.tile([1, w], BF16, tag="osb", name="osb")
                cw = min(512, w)
                for nh in range(max(1, w // 512)):
                    fo = pacc.tile([128, 512], F32, tag="pacc", name="fo")
                    nc.tensor.matmul(fo[0:1, :cw], lhsT=cs["wl2_0"][:],
                                     rhs=zz[0][:, cw * nh:cw * (nh + 1)],
                                     start=True, stop=False)
                    nc.tensor.matmul(fo[0:1, :cw], lhsT=cs["wl2_1"][:],
                                     rhs=zz[1][:, cw * nh:cw * (nh + 1)],
                                     start=False, stop=True)
                    nc.scalar.activation(osb[0:1, cw * nh:cw * (nh + 1)],
                                         fo[0:1, :cw], AF.Identity,
                                         bias=cs["bl2"][0:1, 0:1])
                osf = mp.tile([1, w], F32, tag="osf", name="osf")
                nc.vector.tensor_copy(out=osf[:], in_=osb[:])
                nc.sync.dma_start(out_dram[0:1, lo:lo + w], osf[:])

            for lo in range(0, tcn, 512):
                _phase_c(lo, min(512, tcn - lo))

    nc.finalize()
    return nc


# ---------------------------------------------------------------- host side

def _prep_shared(inp):
    f = lambda k: np.asarray(inp[k], np.float32)
    tok_w, tok_b = f("tok_w"), f("tok_b")
    g1, b1 = f("ln1_g"), f("ln1_b")
    qkv_w, qkv_b = f("qkv_w"), f("qkv_b")
    out_w, out_b = f("out_w"), f("out_b")
    g2, b2 = f("ln2_g"), f("ln2_b")
    ff1_w, ff1_b = f("ff1_w"), f("ff1_b")
    ff2_w, ff2_b = f("ff2_w"), f("ff2_b")

    d = {}
    d["w0xT"] = bfa(tok_w[:, :C].T)
    d["a1"] = bfa(tok_w[:, C:2 * C].T)
    d["a2"] = bfa(tok_w[:, 2 * C:3 * C].T)
    d["a3"] = bfa(tok_w[:, 3 * C:4 * C].T)
    d["tokb_row"] = bfa(tok_b[None, :])

    sc = 1.0 / np.sqrt(DH)
    Wq, Wk, Wv = qkv_w[:C], qkv_w[C:2 * C], qkv_w[2 * C:3 * C]
    bq, bk, bv = qkv_b[:C], qkv_b[C:2 * C], qkv_b[2 * C:3 * C]
    Wq_e = Wq * g1[None, :] * sc
    bq_e = (Wq @ b1) * sc + bq * sc
    Wk_e = Wk * g1[None, :]
    Wv_e = Wv * g1[None, :]
    bv_e = Wv @ b1 + bv
    w_ck = Wk_e.T @ bq_e
    ones = np.ones(C, np.float32)
    d["wqT"] = bfa(Wq_e.T)
    d["wkT"] = bfa(Wk_e.T)
    d["wv_aug"] = bfa(np.concatenate([Wv_e.T, w_ck[:, None]], axis=1))

    for nm, heads in (("woutA", [0, 1, 2, 3]), ("woutB", [4, 5, 6, 7])):
        w = np.zeros((128, 128), np.float32)
        for i, h in enumerate(heads):
            w[32 * i:32 * i + 16, :] = out_w[:, 16 * h:16 * h + 16].T
        d[nm] = bfa(w)
    d["outb_col"] = f32a((out_b + out_w @ bv_e)[:, None])

    for c4 in range(4):
        sl = slice(128 * c4, 128 * (c4 + 1))
        d[f"wf1T_{c4}"] = bfa((ff1_w[sl, :] * g2[None, :]).T)
        d[f"bff1_{c4}"] = f32a((ff1_w[sl, :] @ b2 + ff1_b[sl])[:, None])
        d[f"wf2T_{c4}"] = bfa(ff2_w[:, sl].T)
    d["bff2_col"] = f32a(ff2_b[:, None])

    for nm, wkey, bkey in (("wx1", "xcn_w1", "xcn_b1"),
                           ("wxj1", "xij_w1", "xij_b1")):
        W, B = f(wkey), f(bkey)
        for oc in range(2):
            sl = slice(128 * oc, 128 * (oc + 1))
            d[f"{nm}_{oc}"] = bfa(W[sl, :].T)
            d[f"b{nm[1:]}_{oc}"] = f32a(B[sl][:, None])
    for nm, wkey, bkey in (("wx2", "xcn_w2", "xcn_b2"),
                           ("wx3", "xcn_w3", "xcn_b3"),
                           ("wxj2", "xij_w2", "xij_b2"),
                           ("wl1", "lin_w1", "lin_b1")):
        W, B = f(wkey), f(bkey)
        for ic in range(2):
            for oc in range(2):
                d[f"{nm}_{ic}{oc}"] = bfa(
                    W[128 * oc:128 * (oc + 1), 128 * ic:128 * (ic + 1)].T)
        for oc in range(2):
            d[f"b{nm[1:]}_{oc}"] = f32a(B[128 * oc:128 * (oc + 1)][:, None])
    lin_w2, lin_b2 = f("lin_w2"), f("lin_b2")
    d["wl2_0"] = bfa(lin_w2[0, :128][:, None])
    d["wl2_1"] = bfa(lin_w2[0, 128:][:, None])
    d["bl2"] = f32a(lin_b2.reshape(1, 1))

    Bind = np.zeros((16, 512), np.float32)
    for e in range(16):
        Bind[e, 32 * e:32 * (e + 1)] = 1.0
    d["Bind"] = bfa(Bind)
    bm = np.zeros((128, 1024), np.float32)
    for h in range(8):
        for e in range(4):
            bm[32 * e:32 * (e + 1), 128 * h + 32 * e:128 * h + 32 * (e + 1)] = 1.0
    d["bandmask"] = bfa(bm)
    d["wmean"] = bfa(np.full((128, 1), 1.0 / 128.0))
    d["ones_rep"] = bfa(np.ones((128, 128)))
    d["eps_col"] = f32a(np.full((128, 1), 1e-5))
    d["epsd_col"] = f32a(np.full((128, 1), 1e-30))
    d["beta_col"] = f32a(np.full((128, 1),
                                 np.asarray(inp["beta"],
                                            np.float32).reshape(-1)[0]))
    return d


def _prep_core(inp, core, nst=NST):
    sl = slice(core * TC, (core + 1) * TC)
    tar = np.asarray(inp["tar_ei"])[:, sl].astype(np.int32)
    cols = np.asarray(inp["cn_cols"])[sl].astype(np.int32)     # [TC, K]
    cnt = np.asarray(inp["cn_counts"])[sl].astype(np.int64)    # [TC]

    d = {}
    nsub = TC * K // 128   # 256 subtiles
    d["idx_cn"] = np.ascontiguousarray(
        cols.reshape(-1).reshape(nsub, 128).T)[:, :4 * NST].copy()
    ne = TC // 128
    d["idx_t0"] = np.ascontiguousarray(tar[0].reshape(ne, 128).T)
    d["idx_t1"] = np.ascontiguousarray(tar[1].reshape(ne, 128).T)

    kk = np.arange(K)
    valid = (kk[None, :] < cnt[:, None]).reshape(-1)           # [TC*K]
    vcol = np.ascontiguousarray(valid.reshape(nsub, 128).T)
    d["negv"] = ((~vcol) * np.float32(NEG)).astype(np.float32)[:, :4 * NST].copy()

    ms = valid.astype(np.float32) / np.maximum(cnt, 1).astype(
        np.float32).repeat(K)
    d["msbig"] = np.ascontiguousarray(
        np.broadcast_to(bfa(ms[None, :512 * NST]), (128, 512 * NST)))
    return d


_CACHE = {}
_CACHE_LOCK = threading.Lock()


def _get_nc(nst=NST):
    with _CACHE_LOCK:
        if nst not in _CACHE:
            _CACHE[nst] = _build_nc(nst)
        return _CACHE[nst]


def run(inputs, nst=NST, **spmd_kwargs):
    nc = _get_nc(nst)
    shared = _prep_shared(inputs)
    xbf = np.ascontiguousarray(
        np.asarray(inputs["x"], np.float32)).astype(ml_dtypes.bfloat16)
    in_maps = []
    for core in range(NCORES):
        m = dict(shared)
        m["xbf"] = xbf
        m.update(_prep_core(inputs, core, nst))
        in_maps.append(m)
    res = run_bass_kernel_spmd(nc, in_maps, core_ids=list(range(NCORES)),
                               **spmd_kwargs)
    out = np.stack([res.results[c]["out"][0] for c in range(NCORES)])
    return out, res


def kernel(**inputs):
    out, _ = run(inputs)
    return out.reshape(T, O).astype(np.float32)
